# revision 4
# baseline (speedup 1.0000x reference)
"""Distributed 2-layer GAT (nn_AlignHead) on 8 TRN2 NeuronCores.

Strategy: shard nodes (dst) contiguously across 8 cores. Per core:
  Phase A: LayerNorm + h@W1_ext dense matmul -> per-node table rows
           [g1 (h-major, 512)] bf16 (1024 B — a_src1 is recomputed at the
           destination from the gathered row, a_dst1 stays local).  The
           table is AllGathered in TWO halves (lo = global rows [0,31744) =
           every core's local rows [0,3968); hi = the rest) so the lo
           collective overlaps the tail of phase A.
  Phase B: per dst-tile (128 dsts): dma_gather edge src rows (runs:
           lo-half, hi-half, self-loops via dense local DMA), segment
           softmax via indicator matmuls. P[e,s] built on DVE (is_equal);
           P^T streamed from DRAM (host staircase build). a_dst broadcast
           via P^T-matmul for gather windows; the SELF window skips the
           matmul (z_self = a_src + a_dst tile, pure DVE).  a_src per edge
           = DVE dot of the gathered row with a replicated att_src1 tile.
           p = max(exp(z), exp(0.2 z)), weighted aggregation + denominator
           fused in ONE 520-col matmul pair per window, normalize, ELU ->
           h2, PE transpose, dense h2@W2_ext -> table2 (768-B rows with
           inline a_src2 + ones); two-half AllGather as in phase A.
           Gather EMISSION is software-pipelined (lo-gathers run K tiles
           ahead of hi-gathers) so lo traffic flows while the hi AllGather
           is still landing — the GpSimd queue is in-order.
  Phase C: conv2 edge phase (1 head): attention scalar folded INTO P
           (per-partition scale), single 258-col matmul per window with a
           ones-column denominator; a_dst2 broadcast reuses the SAME
           host-built P^T stream (no PE transposes); normalize -> output.

Self-contained: hardcodes the problem shapes; compiles on first call.
"""
import sys
import types

import numpy as np
import ml_dtypes

# ---------------------------------------------------------------- constants
NCORE = 8
N = 50000
E = 500000
D = 256
H1, C1 = 8, 64
DH = 512            # H1*C1
NEG = 0.2
EPS = 1e-5
NLOC = 6250         # nodes per core
NPAD = 6272         # 49*128
T = 49              # dst tiles per core
LOT = 31            # tiles in the lo half (31*128*8 = 31744 < 32768: int16)
LOROWS = LOT * 128          # 3968 local rows in lo half
HIROWS = NPAD - LOROWS      # 2304 local rows in hi half
ELEM1 = 512         # bf16 elems per conv1 table row (1024 B, h only)
ELEM2 = 384         # bf16 elems per conv2 table row (768 B)
NQ = 4              # swdge queues
BF = ml_dtypes.bfloat16
GCAP = 4            # max windows per dma_gather call (512 descs fit the
                    # SWDGE ring; bigger calls hit superlinear DGE stalls)
SKEW = 4            # lo-gather emission runs SKEW tiles ahead of hi/compute

_cache = {}


def _install_ntff_hook():
    if "antenv.axon_hooks" in sys.modules:
        return
    try:
        import antenv
        mod = types.ModuleType("antenv.axon_hooks")
        _h = [None]
        mod.set_axon_ntff_profile_hook = lambda h: _h.__setitem__(0, h)
        mod.get_axon_ntff_profile_hook = lambda: _h[0]
        sys.modules["antenv.axon_hooks"] = mod
        antenv.axon_hooks = mod
        from trn_agent_boot.trn_boot import _ntff_profile_via_ctypes
        mod.set_axon_ntff_profile_hook(
            _ntff_profile_via_ctypes("/opt/axon/libaxon_pjrt.so"))
    except Exception:
        pass


def _prep_edges(edge_index):
    """Partition + window-pad edges. Runs per tile: 0=lo half, 1=hi half,
    2=self-loops (dense DMA, no gather)."""
    src = np.asarray(edge_index[0]).astype(np.int64)
    dst = np.asarray(edge_index[1]).astype(np.int64)
    loops = np.arange(N, dtype=np.int64)
    src = np.concatenate([src, loops])
    dst = np.concatenate([dst, loops])
    is_self = np.zeros(len(src), bool)
    is_self[E:] = True

    core = dst // NLOC
    ldst = dst % NLOC
    tilei = ldst // 128
    slot = ldst % 128
    s_core = src // NLOC
    s_loc = src % NLOC
    in_lo = s_loc < LOROWS
    srow = np.where(in_lo, s_core * LOROWS + s_loc,
                    s_core * HIROWS + (s_loc - LOROWS))
    run = np.where(is_self, 2, np.where(in_lo, 0, 1))

    nrun = 3
    cnt = np.zeros((NCORE, T, nrun), np.int64)
    np.add.at(cnt, (core, tilei, run), 1)
    NW = np.maximum(1, np.ceil(cnt.max(axis=0) / 128).astype(np.int64))  # [T,nrun]
    nexact = cnt.max(axis=0)             # exact idx count per (t, run)

    Woff = np.zeros((T, nrun), np.int64)
    w = 0
    for t in range(T):
        for r in range(nrun):
            Woff[t, r] = w
            w += NW[t, r]
    Wtot = int(w)

    SWoff = np.zeros((T, nrun), np.int64)
    sw = 0
    for t in range(T):
        sw += sw & 1
        for r in range(nrun):
            SWoff[t, r] = sw
            sw += NW[t, r]
    SWtot = int(sw + (sw & 1))

    order = np.lexsort((ldst, run, tilei, core))
    src_s = srow[order]
    core_s = core[order]
    tile_s = tilei[order]
    run_s = run[order]
    slot_s = slot[order]

    idx_arrs, slot_arrs, pt_arrs = [], [], []
    seg_key = ((core_s * T + tile_s) * nrun + run_s)
    bounds = np.searchsorted(seg_key, np.arange(NCORE * T * nrun + 1))
    srange = np.arange(128)
    NWT1 = int(NW.sum(axis=1).max())
    for c in range(NCORE):
        idx16 = np.zeros((16, Wtot * 8), np.int16)
        slots = np.full((128, SWtot), 128.0, np.float32)
        startv = np.zeros((128, SWtot), np.float32)
        endv = np.zeros((128, SWtot), np.float32)
        for t in range(T):
            for r in range(nrun):
                k = (c * T + t) * nrun + r
                a, b = bounds[k], bounds[k + 1]
                n = b - a
                nw = int(NW[t, r])
                assert n <= nw * 128
                rows = src_s[a:b]
                sl = slot_s[a:b]
                j = np.arange(n)
                w0 = int(Woff[t, r])
                idx16[j % 16, w0 * 8 + j // 16] = rows.astype(np.int16)
                s0 = int(SWoff[t, r])
                slots[j % 128, s0 + j // 128] = sl
                for w in range(nw):
                    wsl = sl[w * 128:(w + 1) * 128]
                    startv[:, s0 + w] = np.searchsorted(wsl, srange, "left")
                    endv[:, s0 + w] = np.searchsorted(wsl, srange, "right")
        idx_arrs.append(np.tile(idx16, (8, 1)))
        slot_arrs.append(slots.astype(BF))
        ptab = np.zeros((T * 128, NWT1 * 128), BF)
        jj = np.arange(128)
        for t in range(T):
            s0 = int(SWoff[t, 0])
            nwt = int(NW[t].sum())
            sv = startv[:, s0:s0 + nwt]
            ev = endv[:, s0:s0 + nwt]
            exp = ((jj[None, None, :] >= sv[:, :, None])
                   & (jj[None, None, :] < ev[:, :, None]))
            ptab[t * 128:(t + 1) * 128, 0:nwt * 128] = \
                exp.reshape(128, nwt * 128).astype(BF)
        pt_arrs.append(ptab)
    return (NW, Woff, SWoff, Wtot, SWtot, nexact,
            idx_arrs, slot_arrs, pt_arrs)


def _build(NW, Woff, SWoff, Wtot, SWtot, nexact, ln_trivial, b1_zero, b2_zero):
    import concourse.bacc as bacc
    import concourse.mybir as mybir
    import concourse.tile as tile

    f32 = mybir.dt.float32
    bf = mybir.dt.bfloat16
    i16 = mybir.dt.int16
    AF = mybir.ActivationFunctionType
    ALU = mybir.AluOpType
    NWT1 = int(NW.sum(axis=1).max())

    nc = bacc.Bacc("TRN2", target_bir_lowering=False, debug=False,
                   num_devices=NCORE, num_swdge_queues=NQ)

    xb_in = nc.declare_dram_parameter("xbf", [NPAD, D], bf, isOutput=False)
    xt_in = nc.declare_dram_parameter("xt", [D, NPAD], bf, isOutput=False)
    ncs_in = nc.declare_dram_parameter("ncs", [128, DH + 8], f32, isOutput=False)
    idx_in = nc.declare_dram_parameter("idx", [128, Wtot * 8], i16, isOutput=False)
    sl_in = nc.declare_dram_parameter("slots", [128, SWtot], bf, isOutput=False)
    pt_in = nc.declare_dram_parameter("ptab", [T * 128, NWT1 * 128], bf,
                                      isOutput=False)
    w1_in = nc.declare_dram_parameter("w1e", [D, DH + 8], bf, isOutput=False)
    w2_in = nc.declare_dram_parameter("w2e", [DH, D + 2], bf, isOutput=False)
    io_in = nc.declare_dram_parameter("iotax", [128, 128 * NWT1], bf, isOutput=False)
    at_in = nc.declare_dram_parameter("att1r", [128, DH], bf, isOutput=False)
    id_in = nc.declare_dram_parameter("ident", [128, 128], bf, isOutput=False)
    badd_in = b1_in = b2_in = None
    if not ln_trivial:
        badd_in = nc.declare_dram_parameter("badd", [128, DH + 8], f32,
                                            isOutput=False)
    if not b1_zero:
        b1_in = nc.declare_dram_parameter("b1r", [128, DH], f32, isOutput=False)
    if not b2_zero:
        b2_in = nc.declare_dram_parameter("b2r", [128, D], f32, isOutput=False)
    out_ext = nc.declare_dram_parameter("out", [NPAD, D], f32, isOutput=True)

    HALFR = [LOROWS, HIROWS]        # local rows per half
    HALFT = [0, LOT, T]             # tile boundaries per half
    tab1_locs = [nc.dram_tensor(f"tab1_loc{q}", [HALFR[q], ELEM1], bf)
                 for q in range(2)]
    tab2_locs = [nc.dram_tensor(f"tab2_loc{q}", [HALFR[q], ELEM2], bf)
                 for q in range(2)]

    qrot = [0]

    def nextq():
        q = qrot[0]
        qrot[0] = (q + 1) % NQ
        return q

    with tile.TileContext(nc) as tc:
        with (
            tc.tile_pool(name="const", bufs=1) as cpool,
            tc.tile_pool(name="dram", bufs=1, space="DRAM") as dpool,
        ):
            tab1_fulls = [dpool.tile([NCORE * HALFR[q], ELEM1], bf,
                                     addr_space="Shared", name=f"t1f{q}")
                          for q in range(2)]
            tab2_fulls = [dpool.tile([NCORE * HALFR[q], ELEM2], bf,
                                     addr_space="Shared", name=f"t2f{q}")
                          for q in range(2)]

            # ---- constants to SBUF
            w1e = cpool.tile([128, 2, DH + 8], bf)
            nc.sync.dma_start(w1e[:], w1_in[:].rearrange("(k p) f -> p k f", p=128))
            w2e = cpool.tile([128, 4, D + 2], bf)
            nc.sync.dma_start(w2e[:], w2_in[:].rearrange("(k p) f -> p k f", p=128))
            iotax = cpool.tile([128, 128 * NWT1], bf)
            nc.sync.dma_start(iotax[:], io_in[:])
            ncs_sb = cpool.tile([128, DH + 8], f32)
            nc.sync.dma_start(ncs_sb[:], ncs_in[:])
            slots_sb = cpool.tile([128, SWtot], bf)
            nc.sync.dma_start(slots_sb[:], sl_in[:])
            idx_sb = cpool.tile([128, Wtot * 8], i16)
            nc.sync.dma_start(idx_sb[:], idx_in[:])
            att1r = cpool.tile([128, DH], bf)
            nc.sync.dma_start(att1r[:], at_in[:])
            ident = cpool.tile([128, 128], bf)
            nc.sync.dma_start(ident[:], id_in[:])
            adst1 = cpool.tile([128, T * 8], bf)
            adst2 = cpool.tile([128, T], bf)
            if not ln_trivial:
                badd_sb = cpool.tile([128, DH + 8], f32)
                nc.sync.dma_start(badd_sb[:], badd_in[:])
            if not b1_zero:
                b1_sb = cpool.tile([128, DH], f32)
                nc.sync.dma_start(b1_sb[:], b1_in[:])
            if not b2_zero:
                b2_sb = cpool.tile([128, D], f32)
                nc.sync.dma_start(b2_sb[:], b2_in[:])

            iotax3 = iotax[:].rearrange("p (s w) -> p s w", w=NWT1)

            # ====== PHASE A: dense1 on raw x with LN folded in afterwards ====
            pha = tc.tile_pool(name="phA", bufs=4)
            iop = pha.__enter__()
            wk_cm = tc.tile_pool(name="wkA", bufs=4)
            wkp = wk_cm.__enter__()
            sm_cm = tc.tile_pool(name="smA", bufs=4)
            smp = sm_cm.__enter__()
            psA_cm = tc.tile_pool(name="psA", bufs=4, space="PSUM")
            psA = psA_cm.__enter__()
            for t in range(T):
                xb = iop.tile([128, D], bf, tag="xb")
                nc.sync.dma_start(xb[:], xb_in[t * 128:(t + 1) * 128, :])
                xT = iop.tile([128, 2, 128], bf, tag="xT")
                nc.sync.dma_start(
                    xT[:], xt_in[:, t * 128:(t + 1) * 128]
                    .rearrange("(k p) n -> p k n", p=128))
                s1 = smp.tile([128, 1], f32, tag="s1")
                nc.vector.reduce_sum(s1[:], xb[:], axis=mybir.AxisListType.X)
                mu = smp.tile([128, 1], f32, tag="mu")
                nc.vector.tensor_scalar_mul(mu[:], s1[:], 1.0 / D)
                sqj = wkp.tile([128, D], f32, tag="sqj")
                s2 = smp.tile([128, 1], f32, tag="s2")
                nc.scalar.activation(sqj[:], xb[:], AF.Square, accum_out=s2[:])
                v1 = smp.tile([128, 1], f32, tag="v1")
                nc.vector.tensor_scalar(v1[:], s2[:], 1.0 / D, EPS, ALU.mult, ALU.add)
                mu2 = smp.tile([128, 1], f32, tag="mu2")
                nc.vector.tensor_mul(mu2[:], mu[:], mu[:])
                var = smp.tile([128, 1], f32, tag="var")
                nc.vector.tensor_tensor(var[:], v1[:], mu2[:], ALU.subtract)
                sd = smp.tile([128, 1], f32, tag="sd")
                nc.scalar.activation(sd[:], var[:], AF.Sqrt)
                rstd = smp.tile([128, 1], f32, tag="rstd")
                nc.vector.reciprocal(rstd[:], sd[:])
                kap = smp.tile([128, 1], f32, tag="kap")
                nc.vector.tensor_mul(kap[:], mu[:], rstd[:])
                ps1 = psA.tile([128, DH], f32, tag="ps1")
                ps1b = psA.tile([128, 8], f32, tag="ps1b")
                for k in range(2):
                    nc.tensor.matmul(ps1[:], xT[:, k, :], w1e[:, k, 0:DH],
                                     start=(k == 0), stop=(k == 1))
                    nc.tensor.matmul(ps1b[:], xT[:, k, :],
                                     w1e[:, k, DH:DH + 8],
                                     start=(k == 0), stop=(k == 1))
                t1 = wkp.tile([128, DH + 8], bf, tag="t1")
                nc.scalar.activation(t1[:, 0:DH], ps1[:], AF.Copy, scale=rstd[:])
                nc.scalar.activation(t1[:, DH:DH + 8], ps1b[:], AF.Copy,
                                     scale=rstd[:])
                tbx = iop.tile([128, DH + 8], bf, tag="tb1")
                nc.vector.scalar_tensor_tensor(
                    tbx[:, 0:DH + 8], ncs_sb[:], kap[:], t1[:],
                    ALU.mult, ALU.add)
                if not ln_trivial:
                    nc.vector.tensor_add(tbx[:, 0:DH + 8], tbx[:, 0:DH + 8],
                                         badd_sb[:])
                nc.scalar.copy(adst1[:, t * 8:(t + 1) * 8],
                               tbx[:, DH:DH + 8])
                qch = 0 if t < LOT else 1
                r0 = t * 128 - (0 if t < LOT else LOROWS)
                nc.sync.dma_start(tab1_locs[qch][r0:r0 + 128, 0:DH],
                                  tbx[:, 0:DH])
                if t == HALFT[qch + 1] - 1:
                    nc.gpsimd.collective_compute(
                        "AllGather", mybir.AluOpType.bypass,
                        replica_groups=[list(range(NCORE))],
                        ins=[tab1_locs[qch][:]],
                        outs=[tab1_fulls[qch].opt()],
                    )

            psA_cm.__exit__(None, None, None)
            sm_cm.__exit__(None, None, None)
            wk_cm.__exit__(None, None, None)
            pha.__exit__(None, None, None)

            # ================= PHASE B: conv1 edges + dense2 =================
            phb = tc.tile_pool(name="phB", bufs=4)
            iop = phb.__enter__()
            wk_cm = tc.tile_pool(name="wkB", bufs=3)
            wkp = wk_cm.__enter__()
            sm_cm = tc.tile_pool(name="smB", bufs=4)
            smp = sm_cm.__enter__()
            ga_cm = tc.tile_pool(name="gaB", bufs=SKEW + 3)
            gap = ga_cm.__enter__()
            st_cm = tc.tile_pool(name="stB", bufs=2)
            stp = st_cm.__enter__()
            psZ_cm = tc.tile_pool(name="psZ", bufs=2, space="PSUM")
            psZ = psZ_cm.__enter__()
            psD_cm = tc.tile_pool(name="psD", bufs=2, space="PSUM")
            psD = psD_cm.__enter__()
            psC_cm = tc.tile_pool(name="psC", bufs=2, space="PSUM")
            psC = psC_cm.__enter__()
            gts = {}
            for tt in range(T + SKEW):
                # lo-gather emission runs SKEW tiles ahead
                if tt < T:
                    gt = gap.tile([128, NWT1, ELEM1], bf, tag="gt1")
                    gts[tt] = gt
                    for (rbase, w0g, w0l, nw, nidx) in _calls(
                            tt, [int(NW[tt, 0])], Woff, nexact):
                        nc.gpsimd.dma_gather(
                            gt[:, w0l:w0l + nw, :], tab1_fulls[0][:],
                            idx_sb[:, w0g * 8:(w0g + nw) * 8],
                            num_idxs=nidx, num_idxs_reg=nidx,
                            elem_size=ELEM1, queue_num=nextq(),
                        )
                if tt < SKEW:
                    continue
                t = tt - SKEW
                gt = gts.pop(t)
                nws = [int(NW[t, r]) for r in range(3)]
                nwt = sum(nws)
                ws = nwt - 1   # self-loop window (dense DMA)
                for (rbase, w0g, w0l, nw, nidx) in _calls_hi(
                        t, nws, Woff, nexact):
                    nc.gpsimd.dma_gather(
                        gt[:, w0l:w0l + nw, :], tab1_fulls[1][:],
                        idx_sb[:, w0g * 8:(w0g + nw) * 8],
                        num_idxs=nidx, num_idxs_reg=nidx,
                        elem_size=ELEM1, queue_num=nextq(),
                    )
                qch = 0 if t < LOT else 1
                r0s = t * 128 - (0 if t < LOT else LOROWS)
                nc.sync.dma_start(gt[:, ws, 0:DH],
                                  tab1_locs[qch][r0s:r0s + 128, 0:DH])
                S0 = int(SWoff[t, 0])
                P = stp.tile([128, 128 * NWT1], bf, tag="P1")
                Pv = P[:, :128 * nwt].rearrange("p (s w) -> p s w", w=nwt)
                nc.vector.tensor_tensor(
                    Pv[:, :, :],
                    slots_sb[:, S0:S0 + nwt].unsqueeze(1)
                    .broadcast_to([128, 128, nwt]),
                    iotax3[:, :, 0:nwt], ALU.is_equal)
                # P^T is pure edge structure — streamed from DRAM (host-built)
                Pt = stp.tile([128, NWT1, 128], bf, tag="Pt1")
                nc.sync.dma_start(
                    Pt[:], pt_in[t * 128:(t + 1) * 128, :]
                    .rearrange("p (w j) -> p w j", j=128))
                stg = stp.tile([128, NWT1, 8 + DH], bf, tag="stg1")
                # a_src per edge: dot(gathered row, att_src1) on DVE; the
                # product is staged in stg's W'' region (overwritten later)
                nc.vector.tensor_mul(
                    stg[:, 0:nwt, 8:8 + DH],
                    gt[:, 0:nwt, 0:DH],
                    att1r[:].unsqueeze(1).broadcast_to([128, nwt, DH]))
                asd = smp.tile([128, NWT1 * 8], f32, tag="asd")
                nc.vector.reduce_sum(
                    asd[:, :nwt * 8].rearrange("p (w h) -> p w h", h=8),
                    stg[:, 0:nwt, 8:8 + DH]
                    .rearrange("p w (h c) -> p w h c", c=C1),
                    axis=mybir.AxisListType.X)
                zb = psZ.tile([128, NWT1 * 8], f32, tag="zb1")
                for w in range(ws):
                    nc.tensor.matmul(zb[:, w * 8:(w + 1) * 8],
                                     Pt[:, w, :],
                                     adst1[:, t * 8:(t + 1) * 8],
                                     start=True, stop=True)
                z = smp.tile([128, NWT1 * 8], f32, tag="z1")
                nc.vector.scalar_tensor_tensor(
                    z[:, :ws * 8], zb[:, :ws * 8], 1.0,
                    asd[:, :ws * 8],
                    ALU.mult, ALU.add)
                # self window: z = a_src(dot) + a_dst(tile) directly on DVE
                nc.vector.tensor_tensor(
                    z[:, ws * 8:nwt * 8], asd[:, ws * 8:nwt * 8],
                    adst1[:, t * 8:(t + 1) * 8], ALU.add)
                e2 = smp.tile([128, NWT1 * 8], f32, tag="e21")
                nc.scalar.activation(e2[:, :nwt * 8], z[:, :nwt * 8], AF.Exp, scale=NEG)
                e1 = smp.tile([128, NWT1 * 8], f32, tag="e11")
                nc.scalar.activation(e1[:, :nwt * 8], z[:, :nwt * 8], AF.Exp)
                nc.vector.tensor_tensor(
                    stg[:, 0:nwt, 0:8],
                    e1[:, :nwt * 8].rearrange("p (w d) -> p w d", d=8),
                    e2[:, :nwt * 8].rearrange("p (w d) -> p w d", d=8),
                    ALU.max)
                # W'' = g (h-major) * p-bcast
                nc.vector.tensor_mul(
                    stg[:, 0:nwt, 8:8 + DH].rearrange("p w (h c) -> p w h c", c=C1),
                    gt[:, 0:nwt, 0:DH].rearrange("p w (h c) -> p w h c", c=C1),
                    stg[:, 0:nwt, 0:8].unsqueeze(3).broadcast_to([128, nwt, 8, C1]))
                # denominator (bank 0, cols 0:8) + numerator (bank 1)
                oc = psC.tile([128, 1024], f32, tag="oc1")
                for w in range(nwt):
                    nc.tensor.matmul(oc[:, 0:8], Pv[:, :, w], stg[:, w, 0:8],
                                     start=(w == 0), stop=(w == nwt - 1))
                    nc.tensor.matmul(oc[:, 512:512 + DH], Pv[:, :, w],
                                     stg[:, w, 8:8 + DH],
                                     start=(w == 0), stop=(w == nwt - 1))
                den = smp.tile([128, 8], f32, tag="den1")
                nc.vector.tensor_scalar_max(den[:], oc[:, 0:8], 1e-30)
                rec = smp.tile([128, 8], f32, tag="rec1")
                nc.vector.reciprocal(rec[:], den[:])
                o1 = wkp.tile([128, DH], bf, tag="o1")
                nc.vector.tensor_tensor(
                    o1[:].rearrange("p (h c) -> p h c", c=C1),
                    oc[:, 512:512 + DH].rearrange("p (h c) -> p h c", c=C1),
                    rec[:].unsqueeze(2).broadcast_to([128, 8, C1]),
                    ALU.mult)
                if not b1_zero:
                    o1f = wkp.tile([128, DH], f32, tag="o1f")
                    nc.vector.tensor_add(o1f[:], o1[:], b1_sb[:])
                    o1 = o1f
                # ELU: h2 = relu(u) + exp(-relu(-u)) - 1
                pos = wkp.tile([128, DH], bf, tag="pos")
                nc.scalar.activation(pos[:], o1[:], AF.Relu)
                rneg = wkp.tile([128, DH], bf, tag="rneg")
                nc.scalar.activation(rneg[:], o1[:], AF.Relu, scale=-1.0)
                en = wkp.tile([128, DH], bf, tag="en")
                nc.scalar.activation(en[:], rneg[:], AF.Exp, scale=-1.0)
                h2 = wkp.tile([128, DH], bf, tag="h2")
                nc.vector.scalar_tensor_tensor(h2[:], pos[:], -1.0, en[:],
                                               ALU.add, ALU.add)
                # dense2
                hT2 = wkp.tile([128, 4, 128], bf, tag="hT2")
                pst = psZ.tile([128, 4, 128], bf, tag="zb1")
                for k in range(4):
                    nc.tensor.transpose(pst[:, k, :], h2[:, k * 128:(k + 1) * 128], ident[:])
                nc.scalar.copy(hT2[:], pst[:])
                ps2 = psD.tile([128, D + 2], f32, tag="ps2")
                for k in range(4):
                    nc.tensor.matmul(ps2[:], hT2[:, k, :], w2e[:, k, :],
                                     start=(k == 0), stop=(k == 3))
                nc.scalar.copy(adst2[:, t:t + 1], ps2[:, D + 1:D + 2])
                tb2 = iop.tile([128, ELEM2], bf, tag="tb2")
                nc.scalar.copy(tb2[:, 0:D + 1], ps2[:, 0:D + 1])
                nc.vector.memset(tb2[:, D + 1:D + 2], 1.0)
                qch = 0 if t < LOT else 1
                r0 = t * 128 - (0 if t < LOT else LOROWS)
                nc.sync.dma_start(tab2_locs[qch][r0:r0 + 128, 0:D + 2],
                                  tb2[:, 0:D + 2])
                if t == HALFT[qch + 1] - 1:
                    nc.gpsimd.collective_compute(
                        "AllGather", mybir.AluOpType.bypass,
                        replica_groups=[list(range(NCORE))],
                        ins=[tab2_locs[qch][:]],
                        outs=[tab2_fulls[qch].opt()],
                    )

            psC_cm.__exit__(None, None, None)
            psD_cm.__exit__(None, None, None)
            psZ_cm.__exit__(None, None, None)
            st_cm.__exit__(None, None, None)
            ga_cm.__exit__(None, None, None)
            sm_cm.__exit__(None, None, None)
            wk_cm.__exit__(None, None, None)
            phb.__exit__(None, None, None)

            # ================= PHASE C: conv2 edges =================
            phc = tc.tile_pool(name="phC", bufs=3)
            iop = phc.__enter__()
            sm_cm = tc.tile_pool(name="smC", bufs=3)
            smp = sm_cm.__enter__()
            ga_cm = tc.tile_pool(name="gaC", bufs=SKEW + 3)
            gap = ga_cm.__enter__()
            st_cm = tc.tile_pool(name="stC", bufs=3)
            stp = st_cm.__enter__()
            psZ_cm = tc.tile_pool(name="psZC", bufs=2, space="PSUM")
            psZ = psZ_cm.__enter__()
            psC_cm = tc.tile_pool(name="psCC", bufs=2, space="PSUM")
            psC = psC_cm.__enter__()
            gts = {}
            for tt in range(T + SKEW):
                if tt < T:
                    gt = gap.tile([128, NWT1, ELEM2], bf, tag="gt2")
                    gts[tt] = gt
                    for (rbase, w0g, w0l, nw, nidx) in _calls(
                            tt, [int(NW[tt, 0])], Woff, nexact):
                        nc.gpsimd.dma_gather(
                            gt[:, w0l:w0l + nw, :], tab2_fulls[0][:],
                            idx_sb[:, w0g * 8:(w0g + nw) * 8],
                            num_idxs=nidx, num_idxs_reg=nidx,
                            elem_size=ELEM2, queue_num=nextq(),
                        )
                if tt < SKEW:
                    continue
                t = tt - SKEW
                gt = gts.pop(t)
                nws = [int(NW[t, r]) for r in range(3)]
                nwt = sum(nws)
                ws = nwt - 1
                for (rbase, w0g, w0l, nw, nidx) in _calls_hi(
                        t, nws, Woff, nexact):
                    nc.gpsimd.dma_gather(
                        gt[:, w0l:w0l + nw, :], tab2_fulls[1][:],
                        idx_sb[:, w0g * 8:(w0g + nw) * 8],
                        num_idxs=nidx, num_idxs_reg=nidx,
                        elem_size=ELEM2, queue_num=nextq(),
                    )
                qch = 0 if t < LOT else 1
                r0s = t * 128 - (0 if t < LOT else LOROWS)
                nc.sync.dma_start(gt[:, ws, 0:D + 2],
                                  tab2_locs[qch][r0s:r0s + 128, 0:D + 2])
                S0 = int(SWoff[t, 0])
                P = stp.tile([128, 128 * NWT1], bf, tag="P2")
                Pv = P[:, :128 * nwt].rearrange("p (s w) -> p s w", w=nwt)
                nc.vector.tensor_tensor(
                    Pv[:, :, :],
                    slots_sb[:, S0:S0 + nwt].unsqueeze(1)
                    .broadcast_to([128, 128, nwt]),
                    iotax3[:, :, 0:nwt], ALU.is_equal)
                # a_dst2 broadcast: reuse the host-built P^T stream
                Pt = stp.tile([128, NWT1, 128], bf, tag="Pt2")
                nc.sync.dma_start(
                    Pt[:], pt_in[t * 128:(t + 1) * 128, :]
                    .rearrange("p (w j) -> p w j", j=128))
                zb = psZ.tile([128, NWT1], f32, tag="zb2")
                for w in range(ws):
                    nc.tensor.matmul(zb[:, w:w + 1],
                                     Pt[:, w, :],
                                     adst2[:, t:t + 1],
                                     start=True, stop=True)
                z = smp.tile([128, NWT1], f32, tag="z2")
                nc.vector.scalar_tensor_tensor(
                    z[:, :ws].rearrange("p (w d) -> p w d", d=1),
                    zb[:, :ws].rearrange("p (w d) -> p w d", d=1), 1.0,
                    gt[:, 0:ws, D:D + 1],
                    ALU.mult, ALU.add)
                nc.vector.tensor_tensor(
                    z[:, ws:nwt], gt[:, ws, D:D + 1],
                    adst2[:, t:t + 1], ALU.add)
                e2 = smp.tile([128, NWT1], f32, tag="e22")
                nc.scalar.activation(e2[:, :nwt], z[:, :nwt], AF.Exp, scale=NEG)
                e1 = smp.tile([128, NWT1], f32, tag="e12")
                nc.scalar.activation(e1[:, :nwt], z[:, :nwt], AF.Exp)
                p2 = smp.tile([128, NWT1], bf, tag="p2")
                nc.vector.tensor_max(p2[:, :nwt], e1[:, :nwt], e2[:, :nwt])
                # fold attention weight into P (1 head): P~ = P * p2[e]
                nc.vector.tensor_tensor(
                    Pv[:, :, :], Pv[:, :, :],
                    p2[:, :nwt].unsqueeze(1).broadcast_to([128, 128, nwt]),
                    ALU.mult)
                # single matmul per window: cols 0:256 numerator, 257 denominator
                oc2 = psC.tile([128, D + 2], f32, tag="oc2")
                for w in range(nwt):
                    nc.tensor.matmul(oc2[:], Pv[:, :, w], gt[:, w, 0:D + 2],
                                     start=(w == 0), stop=(w == nwt - 1))
                den = smp.tile([128, 1], f32, tag="den2")
                nc.vector.tensor_scalar_max(den[:], oc2[:, D + 1:D + 2], 1e-30)
                rec = smp.tile([128, 1], f32, tag="rec2")
                nc.vector.reciprocal(rec[:], den[:])
                outt = iop.tile([128, D], f32, tag="outt")
                nc.vector.tensor_scalar(outt[:], oc2[:, 0:D], rec[:], None, ALU.mult)
                if not b2_zero:
                    nc.vector.tensor_add(outt[:], outt[:], b2_sb[:])
                nc.sync.dma_start(out_ext[t * 128:(t + 1) * 128, :], outt[:])
            psC_cm.__exit__(None, None, None)
            psZ_cm.__exit__(None, None, None)
            st_cm.__exit__(None, None, None)
            ga_cm.__exit__(None, None, None)
            sm_cm.__exit__(None, None, None)
            phc.__exit__(None, None, None)

    nc.compile()
    return nc


def _calls(t, nws, Woff, nexact):
    """Gather call plan for the LO run of tile t:
    (run_base, global_w0, local_w0, nw, nidx)."""
    out = []
    lbase = 0
    for r, nwr in enumerate(nws):
        w0 = int(Woff[t, r])
        ntot = int(nexact[t, r]) if t >= 2 * SKEW else nwr * 128
        done = 0
        while done < nwr:
            nw = min(GCAP, nwr - done)
            nidx = min(nw * 128, max(1, ntot - done * 128))
            out.append((r, w0 + done, lbase + done, nw, nidx))
            done += nw
        lbase += nwr
    return out


def _calls_hi(t, nws, Woff, nexact):
    """Gather call plan for the HI run of tile t (window offsets start after
    the lo run's windows)."""
    out = []
    nwr = nws[1]
    w0 = int(Woff[t, 1])
    lbase = nws[0]
    ntot = int(nexact[t, 1]) if t >= 2 * SKEW else nwr * 128
    done = 0
    while done < nwr:
        nw = min(GCAP, nwr - done)
        nidx = min(nw * 128, max(1, ntot - done * 128))
        out.append((1, w0 + done, lbase + done, nw, nidx))
        done += nw
    return out


def _host_prep(inputs):
    edge_index = np.asarray(inputs["edge_index"])
    x = np.asarray(inputs["x"], np.float32)
    ln_w = np.asarray(inputs["ln_w"], np.float32)
    ln_b = np.asarray(inputs["ln_b"], np.float32)
    W1 = np.asarray(inputs["W1"], np.float32)
    a_s1 = np.asarray(inputs["att_src1"], np.float32)
    a_d1 = np.asarray(inputs["att_dst1"], np.float32)
    b1 = np.asarray(inputs["b1"], np.float32)
    W2 = np.asarray(inputs["W2"], np.float32)
    a_s2 = np.asarray(inputs["att_src2"], np.float32)
    a_d2 = np.asarray(inputs["att_dst2"], np.float32)
    b2 = np.asarray(inputs["b2"], np.float32)

    (NW, Woff, SWoff, Wtot, SWtot, nexact, idx_arrs, slot_arrs,
     pt_arrs) = _prep_edges(edge_index)
    NWT1 = int(NW.sum(axis=1).max())

    ln_trivial = bool(np.all(ln_w == 1.0) and np.all(ln_b == 0.0))

    # W1_ext: natural (h-major) cols + a_dst fold; ln_w folded into rows
    wdst1 = np.stack([W1[:, h * C1:(h + 1) * C1] @ a_d1[h] for h in range(H1)], 1)
    w1o = np.concatenate([W1, wdst1], axis=1)    # [256, 520]
    w1f = ln_w[:, None] * w1o
    w1e = w1f.astype(BF)
    ncs = np.tile((-w1f.sum(axis=0))[None, :], (128, 1)).astype(np.float32)
    badd = np.tile((ln_b @ w1o)[None, :], (128, 1)).astype(np.float32)

    # att_src1 replicated for the dst-side dot (h-major flat layout)
    att1r = np.tile(a_s1.reshape(1, DH), (128, 1)).astype(BF)

    # W2_ext: natural rows (h2 is h-major); cols + att folds
    wsrc2 = W2 @ a_s2[0]
    wdst2 = W2 @ a_d2[0]
    w2e = np.concatenate([W2, wsrc2[:, None], wdst2[:, None]], axis=1).astype(BF)

    iotax = np.zeros((128, 128 * NWT1), np.float32)
    for s in range(128):
        iotax[:, s * NWT1:(s + 1) * NWT1] = s
    iotax = iotax.astype(BF)
    identm = np.eye(128).astype(BF)

    b1_zero = bool(np.all(b1 == 0.0))
    b2_zero = bool(np.all(b2 == 0.0))

    in_maps = []
    for c in range(NCORE):
        xp = np.zeros((NPAD, D), np.float32)
        xp[:NLOC] = x[c * NLOC:(c + 1) * NLOC]
        m = {
            "xbf": xp.astype(BF), "xt": np.ascontiguousarray(xp.T).astype(BF),
            "ncs": ncs, "idx": idx_arrs[c], "slots": slot_arrs[c],
            "ptab": pt_arrs[c],
            "w1e": w1e, "w2e": w2e, "iotax": iotax,
            "att1r": att1r, "ident": identm,
        }
        if not ln_trivial:
            m["badd"] = badd
        if not b1_zero:
            m["b1r"] = np.tile(b1[None, :], (128, 1)).astype(np.float32)
        if not b2_zero:
            m["b2r"] = np.tile(b2[None, :], (128, 1)).astype(np.float32)
        in_maps.append(m)
    meta = (NW, Woff, SWoff, Wtot, SWtot, nexact, ln_trivial, b1_zero, b2_zero)
    return meta, in_maps


def kernel(**inputs):
    _install_ntff_hook()
    from concourse.bass_utils import run_bass_kernel_spmd

    meta, in_maps = _host_prep(inputs)
    NW, Woff, SWoff, Wtot, SWtot, nexact, ln_trivial, b1_zero, b2_zero = meta
    key = (Wtot, SWtot, ln_trivial, b1_zero, b2_zero, NW.tobytes(),
           nexact.tobytes())
    if key not in _cache:
        _cache[key] = _build(NW, Woff, SWoff, Wtot, SWtot, nexact,
                             ln_trivial, b1_zero, b2_zero)
    nc = _cache[key]

    trace = bool(int(__import__("os").environ.get("KERNEL_TRACE", "0")))
    res = run_bass_kernel_spmd(nc, in_maps, core_ids=list(range(NCORE)),
                               trace=trace)
    kernel.last_exec_time_ns = res.exec_time_ns
    out = np.concatenate([res.results[c]["out"][:NLOC] for c in range(NCORE)], 0)
    return out.astype(np.float32)


kernel.last_exec_time_ns = None


# revision 5
# speedup vs baseline: 1.4920x; 1.4920x over previous
"""Distributed 2-layer GAT (nn_AlignHead) on 8 TRN2 NeuronCores.

Strategy: shard nodes (dst) contiguously across 8 cores. Per core:
  Phase A: LayerNorm + h@W1_ext dense matmul -> per-node table rows
           [g1 (c-major, 512) | a_src1 (8) | a_dst1 (8) | pad] bf16.
           The table is AllGathered in TWO halves (lo = global rows
           [0,31744) = every core's local rows [0,3968); hi = the rest),
           each its own Shared tensor (int16 gather indices + exact deps).
  Phase B: per dst-tile (128 dsts): dma_gather edge src rows (runs:
           lo-half, hi-half, self-loops via dense local DMA), segment
           softmax via indicator matmuls.  Software-pipelined three deep:
           lo-gather emission runs SKEW tiles ahead (so lo traffic flows
           while the hi AllGather lands — the GpSimd queue is in-order);
           P/P^T/zb (a_dst broadcast) run one tile ahead of the main
           compute so the PE never head-of-line blocks on the DVE softmax
           chain (which would let the PE HAM throttle back to 1.2 GHz).
           The weighted-aggregation matmuls are chunked 4 windows at a
           time, interleaved with the DVE stg builds.  SELF window skips
           the zb matmul (z_self = a_src_row + a_dst tile, pure DVE).
           ELU -> h2, PE transpose, dense h2@W2_ext -> table2; two-half
           AllGather as in phase A.
  Phase C: conv2 edge phase (1 head): attention scalar folded INTO P
           (per-partition scale), single 258-col matmul per window with a
           ones-column denominator; a_dst2 broadcast reuses the SAME
           host-built P^T stream (no PE transposes); same pipelining.

Self-contained: hardcodes the problem shapes; compiles on first call.
"""
import sys
import types

import numpy as np
import ml_dtypes

# ---------------------------------------------------------------- constants
NCORE = 8
N = 50000
E = 500000
D = 256
H1, C1 = 8, 64
DH = 512            # H1*C1
NEG = 0.2
EPS = 1e-5
NLOC = 6250         # nodes per core
NPAD = 6272         # 49*128
T = 49              # dst tiles per core
LOT = 31            # tiles in the lo half (31*128*8 = 31744 < 32768: int16)
LOROWS = LOT * 128          # 3968 local rows in lo half
HIROWS = NPAD - LOROWS      # 2304 local rows in hi half
ELEM1 = 640         # bf16 elems per conv1 table row (1280 B)
ELEM2 = 384         # bf16 elems per conv2 table row (768 B)
NQ = 4              # swdge queues
BF = ml_dtypes.bfloat16
GCAP = 4            # max windows per dma_gather call
SKEW = 3            # lo-gather emission runs SKEW tiles ahead of compute
WCH = 4             # aggregation window-chunk size

_cache = {}


def _install_ntff_hook():
    if "antenv.axon_hooks" in sys.modules:
        return
    try:
        import antenv
        mod = types.ModuleType("antenv.axon_hooks")
        _h = [None]
        mod.set_axon_ntff_profile_hook = lambda h: _h.__setitem__(0, h)
        mod.get_axon_ntff_profile_hook = lambda: _h[0]
        sys.modules["antenv.axon_hooks"] = mod
        antenv.axon_hooks = mod
        from trn_agent_boot.trn_boot import _ntff_profile_via_ctypes
        mod.set_axon_ntff_profile_hook(
            _ntff_profile_via_ctypes("/opt/axon/libaxon_pjrt.so"))
    except Exception:
        pass


def _prep_edges(edge_index):
    """Partition + window-pad edges. Runs per tile: 0=lo half, 1=hi half,
    2=self-loops (dense DMA, no gather)."""
    src = np.asarray(edge_index[0]).astype(np.int64)
    dst = np.asarray(edge_index[1]).astype(np.int64)
    loops = np.arange(N, dtype=np.int64)
    src = np.concatenate([src, loops])
    dst = np.concatenate([dst, loops])
    is_self = np.zeros(len(src), bool)
    is_self[E:] = True

    core = dst // NLOC
    ldst = dst % NLOC
    tilei = ldst // 128
    slot = ldst % 128
    s_core = src // NLOC
    s_loc = src % NLOC
    in_lo = s_loc < LOROWS
    srow = np.where(in_lo, s_core * LOROWS + s_loc,
                    s_core * HIROWS + (s_loc - LOROWS))
    run = np.where(is_self, 2, np.where(in_lo, 0, 1))

    nrun = 3
    cnt = np.zeros((NCORE, T, nrun), np.int64)
    np.add.at(cnt, (core, tilei, run), 1)
    NW = np.maximum(1, np.ceil(cnt.max(axis=0) / 128).astype(np.int64))  # [T,nrun]
    nexact = cnt.max(axis=0)

    Woff = np.zeros((T, nrun), np.int64)
    w = 0
    for t in range(T):
        for r in range(nrun):
            Woff[t, r] = w
            w += NW[t, r]
    Wtot = int(w)

    SWoff = np.zeros((T, nrun), np.int64)
    sw = 0
    for t in range(T):
        sw += sw & 1
        for r in range(nrun):
            SWoff[t, r] = sw
            sw += NW[t, r]
    SWtot = int(sw + (sw & 1))

    order = np.lexsort((ldst, run, tilei, core))
    src_s = srow[order]
    core_s = core[order]
    tile_s = tilei[order]
    run_s = run[order]
    slot_s = slot[order]

    idx_arrs, slot_arrs, pt_arrs = [], [], []
    seg_key = ((core_s * T + tile_s) * nrun + run_s)
    bounds = np.searchsorted(seg_key, np.arange(NCORE * T * nrun + 1))
    srange = np.arange(128)
    NWT1 = int(NW.sum(axis=1).max())
    for c in range(NCORE):
        idx16 = np.zeros((16, Wtot * 8), np.int16)
        slots = np.full((128, SWtot), 128.0, np.float32)
        startv = np.zeros((128, SWtot), np.float32)
        endv = np.zeros((128, SWtot), np.float32)
        for t in range(T):
            for r in range(nrun):
                k = (c * T + t) * nrun + r
                a, b = bounds[k], bounds[k + 1]
                n = b - a
                nw = int(NW[t, r])
                assert n <= nw * 128
                rows = src_s[a:b]
                sl = slot_s[a:b]
                j = np.arange(n)
                w0 = int(Woff[t, r])
                idx16[j % 16, w0 * 8 + j // 16] = rows.astype(np.int16)
                s0 = int(SWoff[t, r])
                slots[j % 128, s0 + j // 128] = sl
                for w in range(nw):
                    wsl = sl[w * 128:(w + 1) * 128]
                    startv[:, s0 + w] = np.searchsorted(wsl, srange, "left")
                    endv[:, s0 + w] = np.searchsorted(wsl, srange, "right")
        idx_arrs.append(np.tile(idx16, (8, 1)))
        slot_arrs.append(slots.astype(BF))
        ptab = np.zeros((T * 128, NWT1 * 128), BF)
        jj = np.arange(128)
        for t in range(T):
            s0 = int(SWoff[t, 0])
            nwt = int(NW[t].sum())
            sv = startv[:, s0:s0 + nwt]
            ev = endv[:, s0:s0 + nwt]
            exp = ((jj[None, None, :] >= sv[:, :, None])
                   & (jj[None, None, :] < ev[:, :, None]))
            ptab[t * 128:(t + 1) * 128, 0:nwt * 128] = \
                exp.reshape(128, nwt * 128).astype(BF)
        pt_arrs.append(ptab)
    return (NW, Woff, SWoff, Wtot, SWtot, nexact,
            idx_arrs, slot_arrs, pt_arrs)


def _build(NW, Woff, SWoff, Wtot, SWtot, nexact, ln_trivial, b1_zero, b2_zero):
    import concourse.bacc as bacc
    import concourse.mybir as mybir
    import concourse.tile as tile

    f32 = mybir.dt.float32
    bf = mybir.dt.bfloat16
    i16 = mybir.dt.int16
    AF = mybir.ActivationFunctionType
    ALU = mybir.AluOpType
    NWT1 = int(NW.sum(axis=1).max())

    nc = bacc.Bacc("TRN2", target_bir_lowering=False, debug=False,
                   num_devices=NCORE, num_swdge_queues=NQ)

    xb_in = nc.declare_dram_parameter("xbf", [NPAD, D], bf, isOutput=False)
    xt_in = nc.declare_dram_parameter("xt", [D, NPAD], bf, isOutput=False)
    ncs_in = nc.declare_dram_parameter("ncs", [128, DH + 16], f32, isOutput=False)
    idx_in = nc.declare_dram_parameter("idx", [128, Wtot * 8], i16, isOutput=False)
    sl_in = nc.declare_dram_parameter("slots", [128, SWtot], bf, isOutput=False)
    pt_in = nc.declare_dram_parameter("ptab", [T * 128, NWT1 * 128], bf,
                                      isOutput=False)
    w1_in = nc.declare_dram_parameter("w1e", [D, DH + 16], bf, isOutput=False)
    w2_in = nc.declare_dram_parameter("w2e", [DH, D + 2], bf, isOutput=False)
    io_in = nc.declare_dram_parameter("iotax", [128, 128 * NWT1], bf, isOutput=False)
    id_in = nc.declare_dram_parameter("ident", [128, 128], bf, isOutput=False)
    badd_in = b1_in = b2_in = None
    if not ln_trivial:
        badd_in = nc.declare_dram_parameter("badd", [128, DH + 16], f32,
                                            isOutput=False)
    if not b1_zero:
        b1_in = nc.declare_dram_parameter("b1r", [128, DH], f32, isOutput=False)
    if not b2_zero:
        b2_in = nc.declare_dram_parameter("b2r", [128, D], f32, isOutput=False)
    out_ext = nc.declare_dram_parameter("out", [NPAD, D], f32, isOutput=True)

    HALFR = [LOROWS, HIROWS]
    HALFT = [0, LOT, T]
    tab1_locs = [nc.dram_tensor(f"tab1_loc{q}", [HALFR[q], ELEM1], bf)
                 for q in range(2)]
    tab2_locs = [nc.dram_tensor(f"tab2_loc{q}", [HALFR[q], ELEM2], bf)
                 for q in range(2)]

    qrot = [0]

    def nextq():
        q = qrot[0]
        qrot[0] = (q + 1) % NQ
        return q

    with tile.TileContext(nc) as tc:
        with (
            tc.tile_pool(name="const", bufs=1) as cpool,
            tc.tile_pool(name="dram", bufs=1, space="DRAM") as dpool,
        ):
            tab1_fulls = [dpool.tile([NCORE * HALFR[q], ELEM1], bf,
                                     addr_space="Shared", name=f"t1f{q}")
                          for q in range(2)]
            tab2_fulls = [dpool.tile([NCORE * HALFR[q], ELEM2], bf,
                                     addr_space="Shared", name=f"t2f{q}")
                          for q in range(2)]

            # ---- constants to SBUF
            w1e = cpool.tile([128, 2, DH + 16], bf)
            nc.sync.dma_start(w1e[:], w1_in[:].rearrange("(k p) f -> p k f", p=128))
            w2e = cpool.tile([128, 4, D + 2], bf)
            nc.sync.dma_start(w2e[:], w2_in[:].rearrange("(k p) f -> p k f", p=128))
            iotax = cpool.tile([128, 128 * NWT1], bf)
            nc.sync.dma_start(iotax[:], io_in[:])
            ncs_sb = cpool.tile([128, DH + 16], f32)
            nc.sync.dma_start(ncs_sb[:], ncs_in[:])
            ident = cpool.tile([128, 128], bf)
            nc.sync.dma_start(ident[:], id_in[:])
            slots_sb = cpool.tile([128, SWtot], bf)
            nc.sync.dma_start(slots_sb[:], sl_in[:])
            idx_sb = cpool.tile([128, Wtot * 8], i16)
            nc.sync.dma_start(idx_sb[:], idx_in[:])
            adst1 = cpool.tile([128, T * 8], bf)
            adst2 = cpool.tile([128, T], bf)
            if not ln_trivial:
                badd_sb = cpool.tile([128, DH + 16], f32)
                nc.sync.dma_start(badd_sb[:], badd_in[:])
            if not b1_zero:
                b1_sb = cpool.tile([128, DH], f32)
                nc.sync.dma_start(b1_sb[:], b1_in[:])
            if not b2_zero:
                b2_sb = cpool.tile([128, D], f32)
                nc.sync.dma_start(b2_sb[:], b2_in[:])

            iotax3 = iotax[:].rearrange("p (s w) -> p s w", w=NWT1)

            # ====== PHASE A ======
            pha = tc.tile_pool(name="phA", bufs=4)
            iop = pha.__enter__()
            wk_cm = tc.tile_pool(name="wkA", bufs=4)
            wkp = wk_cm.__enter__()
            sm_cm = tc.tile_pool(name="smA", bufs=4)
            smp = sm_cm.__enter__()
            psA_cm = tc.tile_pool(name="psA", bufs=4, space="PSUM")
            psA = psA_cm.__enter__()
            for t in range(T):
                xb = iop.tile([128, D], bf, tag="xb")
                nc.sync.dma_start(xb[:], xb_in[t * 128:(t + 1) * 128, :])
                xT = iop.tile([128, 2, 128], bf, tag="xT")
                nc.sync.dma_start(
                    xT[:], xt_in[:, t * 128:(t + 1) * 128]
                    .rearrange("(k p) n -> p k n", p=128))
                s1 = smp.tile([128, 1], f32, tag="s1")
                nc.vector.reduce_sum(s1[:], xb[:], axis=mybir.AxisListType.X)
                mu = smp.tile([128, 1], f32, tag="mu")
                nc.vector.tensor_scalar_mul(mu[:], s1[:], 1.0 / D)
                sqj = wkp.tile([128, D], f32, tag="sqj")
                s2 = smp.tile([128, 1], f32, tag="s2")
                nc.scalar.activation(sqj[:], xb[:], AF.Square, accum_out=s2[:])
                v1 = smp.tile([128, 1], f32, tag="v1")
                nc.vector.tensor_scalar(v1[:], s2[:], 1.0 / D, EPS, ALU.mult, ALU.add)
                mu2 = smp.tile([128, 1], f32, tag="mu2")
                nc.vector.tensor_mul(mu2[:], mu[:], mu[:])
                var = smp.tile([128, 1], f32, tag="var")
                nc.vector.tensor_tensor(var[:], v1[:], mu2[:], ALU.subtract)
                sd = smp.tile([128, 1], f32, tag="sd")
                nc.scalar.activation(sd[:], var[:], AF.Sqrt)
                rstd = smp.tile([128, 1], f32, tag="rstd")
                nc.vector.reciprocal(rstd[:], sd[:])
                kap = smp.tile([128, 1], f32, tag="kap")
                nc.vector.tensor_mul(kap[:], mu[:], rstd[:])
                ps1 = psA.tile([128, DH], f32, tag="ps1")
                ps1b = psA.tile([128, 16], f32, tag="ps1b")
                for k in range(2):
                    nc.tensor.matmul(ps1[:], xT[:, k, :], w1e[:, k, 0:DH],
                                     start=(k == 0), stop=(k == 1))
                    nc.tensor.matmul(ps1b[:], xT[:, k, :],
                                     w1e[:, k, DH:DH + 16],
                                     start=(k == 0), stop=(k == 1))
                t1 = wkp.tile([128, DH + 16], bf, tag="t1")
                nc.scalar.activation(t1[:, 0:DH], ps1[:], AF.Copy, scale=rstd[:])
                nc.scalar.activation(t1[:, DH:DH + 16], ps1b[:], AF.Copy,
                                     scale=rstd[:])
                tbx = iop.tile([128, DH + 16], bf, tag="tb1")
                nc.vector.scalar_tensor_tensor(
                    tbx[:, 0:DH + 16], ncs_sb[:], kap[:], t1[:],
                    ALU.mult, ALU.add)
                if not ln_trivial:
                    nc.vector.tensor_add(tbx[:, 0:DH + 16], tbx[:, 0:DH + 16],
                                         badd_sb[:])
                nc.scalar.copy(adst1[:, t * 8:(t + 1) * 8],
                               tbx[:, DH + 8:DH + 16])
                qch = 0 if t < LOT else 1
                r0 = t * 128 - (0 if t < LOT else LOROWS)
                nc.sync.dma_start(tab1_locs[qch][r0:r0 + 128, 0:DH + 8],
                                  tbx[:, 0:DH + 8])
                if t == HALFT[qch + 1] - 1:
                    nc.gpsimd.collective_compute(
                        "AllGather", mybir.AluOpType.bypass,
                        replica_groups=[list(range(NCORE))],
                        ins=[tab1_locs[qch][:]],
                        outs=[tab1_fulls[qch].opt()],
                    )

            psA_cm.__exit__(None, None, None)
            sm_cm.__exit__(None, None, None)
            wk_cm.__exit__(None, None, None)
            pha.__exit__(None, None, None)

            # ================= PHASE B: conv1 edges + dense2 =================
            # 3-deep software pipeline:
            #   stage 0 (tile tt):      lo-gather emission
            #   stage 1 (tile t1=tt-SKEW+1): hi-gather, self DMA, Pt stream,
            #                                P build, zb matmuls
            #   stage 2 (tile t2=tt-SKEW):   softmax + chunked aggregation +
            #                                dense2 + table write + AG
            phb = tc.tile_pool(name="phB", bufs=4)
            iop = phb.__enter__()
            wk_cm = tc.tile_pool(name="wkB", bufs=3)
            wkp = wk_cm.__enter__()
            sm_cm = tc.tile_pool(name="smB", bufs=4)
            smp = sm_cm.__enter__()
            ga_cm = tc.tile_pool(name="gaB", bufs=SKEW + 3)
            gap = ga_cm.__enter__()
            st_cm = tc.tile_pool(name="stB", bufs=2)
            stp = st_cm.__enter__()
            psZ_cm = tc.tile_pool(name="psZ", bufs=3, space="PSUM")
            psZ = psZ_cm.__enter__()
            psD_cm = tc.tile_pool(name="psD", bufs=1, space="PSUM")
            psD = psD_cm.__enter__()
            psC_cm = tc.tile_pool(name="psC", bufs=2, space="PSUM")
            psC = psC_cm.__enter__()
            gts, Ps, Pts, zbs = {}, {}, {}, {}
            for tt in range(T + SKEW):
                if tt < T:
                    gt = gap.tile([128, NWT1, ELEM1], bf, tag="gt1")
                    gts[tt] = gt
                    for (rbase, w0g, w0l, nw, nidx) in _calls(
                            tt, 0, 0, Woff, NW, nexact):
                        nc.gpsimd.dma_gather(
                            gt[:, w0l:w0l + nw, :], tab1_fulls[0][:],
                            idx_sb[:, w0g * 8:(w0g + nw) * 8],
                            num_idxs=nidx, num_idxs_reg=nidx,
                            elem_size=ELEM1, queue_num=nextq(),
                        )
                t1 = tt - SKEW + 1
                if 0 <= t1 < T:
                    gt = gts[t1]
                    nws = [int(NW[t1, r]) for r in range(3)]
                    nwt1 = sum(nws)
                    ws1 = nwt1 - 1
                    for (rbase, w0g, w0l, nw, nidx) in _calls(
                            t1, 1, nws[0], Woff, NW, nexact):
                        nc.gpsimd.dma_gather(
                            gt[:, w0l:w0l + nw, :], tab1_fulls[1][:],
                            idx_sb[:, w0g * 8:(w0g + nw) * 8],
                            num_idxs=nidx, num_idxs_reg=nidx,
                            elem_size=ELEM1, queue_num=nextq(),
                        )
                    qch = 0 if t1 < LOT else 1
                    r0s = t1 * 128 - (0 if t1 < LOT else LOROWS)
                    nc.sync.dma_start(gt[:, ws1, 0:DH + 8],
                                      tab1_locs[qch][r0s:r0s + 128, 0:DH + 8])
                    S0 = int(SWoff[t1, 0])
                    P = stp.tile([128, 128 * NWT1], bf, tag="P1")
                    Ps[t1] = P
                    Pv = P[:, :128 * nwt1].rearrange("p (s w) -> p s w", w=nwt1)
                    nc.vector.tensor_tensor(
                        Pv[:, :, :],
                        slots_sb[:, S0:S0 + nwt1].unsqueeze(1)
                        .broadcast_to([128, 128, nwt1]),
                        iotax3[:, :, 0:nwt1], ALU.is_equal)
                    Pt = stp.tile([128, NWT1, 128], bf, tag="Pt1")
                    Pts[t1] = Pt
                    nc.sync.dma_start(
                        Pt[:], pt_in[t1 * 128:(t1 + 1) * 128, :]
                        .rearrange("p (w j) -> p w j", j=128))
                    zb = psZ.tile([128, NWT1 * 8], f32, tag="zbt")
                    zbs[t1] = zb
                    for w in range(ws1):
                        nc.tensor.matmul(zb[:, w * 8:(w + 1) * 8],
                                         Pt[:, w, :],
                                         adst1[:, t1 * 8:(t1 + 1) * 8],
                                         start=True, stop=True)
                if tt < SKEW:
                    continue
                t = tt - SKEW
                gt = gts.pop(t)
                P = Ps.pop(t)
                Pt = Pts.pop(t)
                zb = zbs.pop(t)
                nws = [int(NW[t, r]) for r in range(3)]
                nwt = sum(nws)
                ws = nwt - 1
                Pv = P[:, :128 * nwt].rearrange("p (s w) -> p s w", w=nwt)
                z = smp.tile([128, NWT1 * 8], f32, tag="z1")
                nc.vector.scalar_tensor_tensor(
                    z[:, :ws * 8].rearrange("p (w d) -> p w d", d=8),
                    zb[:, :ws * 8].rearrange("p (w d) -> p w d", d=8), 1.0,
                    gt[:, 0:ws, DH:DH + 8],
                    ALU.mult, ALU.add)
                nc.vector.tensor_tensor(
                    z[:, ws * 8:nwt * 8], gt[:, ws, DH:DH + 8],
                    adst1[:, t * 8:(t + 1) * 8], ALU.add)
                e2 = smp.tile([128, NWT1 * 8], f32, tag="e21")
                nc.scalar.activation(e2[:, :nwt * 8], z[:, :nwt * 8], AF.Exp, scale=NEG)
                e1 = smp.tile([128, NWT1 * 8], f32, tag="e11")
                nc.scalar.activation(e1[:, :nwt * 8], z[:, :nwt * 8], AF.Exp)
                stg = stp.tile([128, NWT1, 8 + DH], bf, tag="stg1")
                oc = psC.tile([128, 1024], f32, tag="oc1")
                # chunked: DVE builds p/W'' for WCH windows, PE aggregates
                # them while DVE builds the next chunk
                for c0 in range(0, nwt, WCH):
                    c1 = min(c0 + WCH, nwt)
                    nc.vector.tensor_tensor(
                        stg[:, c0:c1, 0:8],
                        e1[:, c0 * 8:c1 * 8].rearrange("p (w d) -> p w d", d=8),
                        e2[:, c0 * 8:c1 * 8].rearrange("p (w d) -> p w d", d=8),
                        ALU.max)
                    nc.vector.tensor_mul(
                        stg[:, c0:c1, 8:8 + DH].rearrange(
                            "p w (c h) -> p w c h", h=8),
                        gt[:, c0:c1, 0:DH].rearrange("p w (c h) -> p w c h", h=8),
                        stg[:, c0:c1, 0:8].unsqueeze(2)
                        .broadcast_to([128, c1 - c0, 64, 8]))
                    for w in range(c0, c1):
                        nc.tensor.matmul(oc[:, 0:8], Pv[:, :, w], stg[:, w, 0:8],
                                         start=(w == 0), stop=(w == nwt - 1))
                        nc.tensor.matmul(oc[:, 512:512 + DH], Pv[:, :, w],
                                         stg[:, w, 8:8 + DH],
                                         start=(w == 0), stop=(w == nwt - 1))
                den = smp.tile([128, 8], f32, tag="den1")
                nc.vector.tensor_scalar_max(den[:], oc[:, 0:8], 1e-30)
                rec = smp.tile([128, 8], f32, tag="rec1")
                nc.vector.reciprocal(rec[:], den[:])
                o1 = wkp.tile([128, DH], bf, tag="o1")
                nc.vector.tensor_tensor(
                    o1[:].rearrange("p (c h) -> p c h", h=8),
                    oc[:, 512:512 + DH].rearrange("p (c h) -> p c h", h=8),
                    rec[:].unsqueeze(1).broadcast_to([128, 64, 8]),
                    ALU.mult)
                if not b1_zero:
                    o1f = wkp.tile([128, DH], f32, tag="o1f")
                    nc.vector.tensor_add(o1f[:], o1[:], b1_sb[:])
                    o1 = o1f
                pos = wkp.tile([128, DH], bf, tag="pos")
                nc.scalar.activation(pos[:], o1[:], AF.Relu)
                rneg = wkp.tile([128, DH], bf, tag="rneg")
                nc.scalar.activation(rneg[:], o1[:], AF.Relu, scale=-1.0)
                en = wkp.tile([128, DH], bf, tag="en")
                nc.scalar.activation(en[:], rneg[:], AF.Exp, scale=-1.0)
                h2 = wkp.tile([128, DH], bf, tag="h2")
                nc.vector.scalar_tensor_tensor(h2[:], pos[:], -1.0, en[:],
                                               ALU.add, ALU.add)
                hT2 = wkp.tile([128, 4, 128], bf, tag="hT2")
                pst = psZ.tile([128, 4, 128], bf, tag="zbt")
                for k in range(4):
                    nc.tensor.transpose(pst[:, k, :], h2[:, k * 128:(k + 1) * 128], ident[:])
                nc.scalar.copy(hT2[:], pst[:])
                ps2 = psD.tile([128, D + 2], f32, tag="ps2")
                for k in range(4):
                    nc.tensor.matmul(ps2[:], hT2[:, k, :], w2e[:, k, :],
                                     start=(k == 0), stop=(k == 3))
                nc.scalar.copy(adst2[:, t:t + 1], ps2[:, D + 1:D + 2])
                tb2 = iop.tile([128, ELEM2], bf, tag="tb2")
                nc.scalar.copy(tb2[:, 0:D + 1], ps2[:, 0:D + 1])
                nc.vector.memset(tb2[:, D + 1:D + 2], 1.0)
                qch = 0 if t < LOT else 1
                r0 = t * 128 - (0 if t < LOT else LOROWS)
                nc.sync.dma_start(tab2_locs[qch][r0:r0 + 128, 0:D + 2],
                                  tb2[:, 0:D + 2])
                if t == HALFT[qch + 1] - 1:
                    nc.gpsimd.collective_compute(
                        "AllGather", mybir.AluOpType.bypass,
                        replica_groups=[list(range(NCORE))],
                        ins=[tab2_locs[qch][:]],
                        outs=[tab2_fulls[qch].opt()],
                    )

            psC_cm.__exit__(None, None, None)
            psD_cm.__exit__(None, None, None)
            psZ_cm.__exit__(None, None, None)
            st_cm.__exit__(None, None, None)
            ga_cm.__exit__(None, None, None)
            sm_cm.__exit__(None, None, None)
            wk_cm.__exit__(None, None, None)
            phb.__exit__(None, None, None)

            # ================= PHASE C: conv2 edges =================
            phc = tc.tile_pool(name="phC", bufs=3)
            iop = phc.__enter__()
            sm_cm = tc.tile_pool(name="smC", bufs=3)
            smp = sm_cm.__enter__()
            ga_cm = tc.tile_pool(name="gaC", bufs=SKEW + 3)
            gap = ga_cm.__enter__()
            st_cm = tc.tile_pool(name="stC", bufs=2)
            stp = st_cm.__enter__()
            psZ_cm = tc.tile_pool(name="psZC", bufs=2, space="PSUM")
            psZ = psZ_cm.__enter__()
            psC_cm = tc.tile_pool(name="psCC", bufs=2, space="PSUM")
            psC = psC_cm.__enter__()
            gts, Ps, Pts, zbs = {}, {}, {}, {}
            for tt in range(T + SKEW):
                if tt < T:
                    gt = gap.tile([128, NWT1, ELEM2], bf, tag="gt2")
                    gts[tt] = gt
                    for (rbase, w0g, w0l, nw, nidx) in _calls(
                            tt, 0, 0, Woff, NW, nexact):
                        nc.gpsimd.dma_gather(
                            gt[:, w0l:w0l + nw, :], tab2_fulls[0][:],
                            idx_sb[:, w0g * 8:(w0g + nw) * 8],
                            num_idxs=nidx, num_idxs_reg=nidx,
                            elem_size=ELEM2, queue_num=nextq(),
                        )
                t1 = tt - SKEW + 1
                if 0 <= t1 < T:
                    gt = gts[t1]
                    nws = [int(NW[t1, r]) for r in range(3)]
                    nwt1 = sum(nws)
                    ws1 = nwt1 - 1
                    for (rbase, w0g, w0l, nw, nidx) in _calls(
                            t1, 1, nws[0], Woff, NW, nexact):
                        nc.gpsimd.dma_gather(
                            gt[:, w0l:w0l + nw, :], tab2_fulls[1][:],
                            idx_sb[:, w0g * 8:(w0g + nw) * 8],
                            num_idxs=nidx, num_idxs_reg=nidx,
                            elem_size=ELEM2, queue_num=nextq(),
                        )
                    qch = 0 if t1 < LOT else 1
                    r0s = t1 * 128 - (0 if t1 < LOT else LOROWS)
                    nc.sync.dma_start(gt[:, ws1, 0:D + 2],
                                      tab2_locs[qch][r0s:r0s + 128, 0:D + 2])
                    S0 = int(SWoff[t1, 0])
                    P = stp.tile([128, 128 * NWT1], bf, tag="P2")
                    Ps[t1] = P
                    Pv = P[:, :128 * nwt1].rearrange("p (s w) -> p s w", w=nwt1)
                    nc.vector.tensor_tensor(
                        Pv[:, :, :],
                        slots_sb[:, S0:S0 + nwt1].unsqueeze(1)
                        .broadcast_to([128, 128, nwt1]),
                        iotax3[:, :, 0:nwt1], ALU.is_equal)
                    Pt = stp.tile([128, NWT1, 128], bf, tag="Pt2")
                    Pts[t1] = Pt
                    nc.sync.dma_start(
                        Pt[:], pt_in[t1 * 128:(t1 + 1) * 128, :]
                        .rearrange("p (w j) -> p w j", j=128))
                    zb = psZ.tile([128, NWT1], f32, tag="zb2")
                    zbs[t1] = zb
                    for w in range(ws1):
                        nc.tensor.matmul(zb[:, w:w + 1],
                                         Pt[:, w, :],
                                         adst2[:, t1:t1 + 1],
                                         start=True, stop=True)
                if tt < SKEW:
                    continue
                t = tt - SKEW
                gt = gts.pop(t)
                P = Ps.pop(t)
                Pt = Pts.pop(t)
                zb = zbs.pop(t)
                nws = [int(NW[t, r]) for r in range(3)]
                nwt = sum(nws)
                ws = nwt - 1
                Pv = P[:, :128 * nwt].rearrange("p (s w) -> p s w", w=nwt)
                z = smp.tile([128, NWT1], f32, tag="z2")
                nc.vector.scalar_tensor_tensor(
                    z[:, :ws].rearrange("p (w d) -> p w d", d=1),
                    zb[:, :ws].rearrange("p (w d) -> p w d", d=1), 1.0,
                    gt[:, 0:ws, D:D + 1],
                    ALU.mult, ALU.add)
                nc.vector.tensor_tensor(
                    z[:, ws:nwt], gt[:, ws, D:D + 1],
                    adst2[:, t:t + 1], ALU.add)
                e2 = smp.tile([128, NWT1], f32, tag="e22")
                nc.scalar.activation(e2[:, :nwt], z[:, :nwt], AF.Exp, scale=NEG)
                e1 = smp.tile([128, NWT1], f32, tag="e12")
                nc.scalar.activation(e1[:, :nwt], z[:, :nwt], AF.Exp)
                p2 = smp.tile([128, NWT1], bf, tag="p2")
                nc.vector.tensor_max(p2[:, :nwt], e1[:, :nwt], e2[:, :nwt])
                oc2 = psC.tile([128, D + 2], f32, tag="oc2")
                for c0 in range(0, nwt, WCH):
                    c1 = min(c0 + WCH, nwt)
                    nc.vector.tensor_tensor(
                        Pv[:, :, c0:c1], Pv[:, :, c0:c1],
                        p2[:, c0:c1].unsqueeze(1)
                        .broadcast_to([128, 128, c1 - c0]),
                        ALU.mult)
                    for w in range(c0, c1):
                        nc.tensor.matmul(oc2[:], Pv[:, :, w], gt[:, w, 0:D + 2],
                                         start=(w == 0), stop=(w == nwt - 1))
                den = smp.tile([128, 1], f32, tag="den2")
                nc.vector.tensor_scalar_max(den[:], oc2[:, D + 1:D + 2], 1e-30)
                rec = smp.tile([128, 1], f32, tag="rec2")
                nc.vector.reciprocal(rec[:], den[:])
                outt = iop.tile([128, D], f32, tag="outt")
                nc.vector.tensor_scalar(outt[:], oc2[:, 0:D], rec[:], None, ALU.mult)
                if not b2_zero:
                    nc.vector.tensor_add(outt[:], outt[:], b2_sb[:])
                nc.sync.dma_start(out_ext[t * 128:(t + 1) * 128, :], outt[:])
            psC_cm.__exit__(None, None, None)
            psZ_cm.__exit__(None, None, None)
            st_cm.__exit__(None, None, None)
            ga_cm.__exit__(None, None, None)
            sm_cm.__exit__(None, None, None)
            phc.__exit__(None, None, None)

    nc.compile()
    return nc


def _calls(t, r, lbase, Woff, NW, nexact):
    """Gather call plan for run r of tile t:
    (run, global_w0, local_w0, nw, nidx)."""
    out = []
    nwr = int(NW[t, r])
    w0 = int(Woff[t, r])
    ntot = int(nexact[t, r]) if t >= 2 * (SKEW + 1) else nwr * 128
    done = 0
    while done < nwr:
        nw = min(GCAP, nwr - done)
        nidx = min(nw * 128, max(1, ntot - done * 128))
        out.append((r, w0 + done, lbase + done, nw, nidx))
        done += nw
    return out


def _host_prep(inputs):
    edge_index = np.asarray(inputs["edge_index"])
    x = np.asarray(inputs["x"], np.float32)
    ln_w = np.asarray(inputs["ln_w"], np.float32)
    ln_b = np.asarray(inputs["ln_b"], np.float32)
    W1 = np.asarray(inputs["W1"], np.float32)
    a_s1 = np.asarray(inputs["att_src1"], np.float32)
    a_d1 = np.asarray(inputs["att_dst1"], np.float32)
    b1 = np.asarray(inputs["b1"], np.float32)
    W2 = np.asarray(inputs["W2"], np.float32)
    a_s2 = np.asarray(inputs["att_src2"], np.float32)
    a_d2 = np.asarray(inputs["att_dst2"], np.float32)
    b2 = np.asarray(inputs["b2"], np.float32)

    (NW, Woff, SWoff, Wtot, SWtot, nexact, idx_arrs, slot_arrs,
     pt_arrs) = _prep_edges(edge_index)
    NWT1 = int(NW.sum(axis=1).max())

    ln_trivial = bool(np.all(ln_w == 1.0) and np.all(ln_b == 0.0))

    perm1 = np.empty(DH, np.int64)
    for h in range(H1):
        for c in range(C1):
            perm1[c * 8 + h] = h * C1 + c
    wsrc1 = np.stack([W1[:, h * C1:(h + 1) * C1] @ a_s1[h] for h in range(H1)], 1)
    wdst1 = np.stack([W1[:, h * C1:(h + 1) * C1] @ a_d1[h] for h in range(H1)], 1)
    w1o = np.concatenate([W1[:, perm1], wsrc1, wdst1], axis=1)    # [256, 528]
    w1f = ln_w[:, None] * w1o
    w1e = w1f.astype(BF)
    ncs = np.tile((-w1f.sum(axis=0))[None, :], (128, 1)).astype(np.float32)
    badd = np.tile((ln_b @ w1o)[None, :], (128, 1)).astype(np.float32)

    W2r = W2[perm1, :]
    wsrc2 = W2r @ a_s2[0]
    wdst2 = W2r @ a_d2[0]
    w2e = np.concatenate([W2r, wsrc2[:, None], wdst2[:, None]], axis=1).astype(BF)

    iotax = np.zeros((128, 128 * NWT1), np.float32)
    for s in range(128):
        iotax[:, s * NWT1:(s + 1) * NWT1] = s
    iotax = iotax.astype(BF)
    identm = np.eye(128).astype(BF)

    b1_zero = bool(np.all(b1 == 0.0))
    b2_zero = bool(np.all(b2 == 0.0))

    in_maps = []
    for c in range(NCORE):
        xp = np.zeros((NPAD, D), np.float32)
        xp[:NLOC] = x[c * NLOC:(c + 1) * NLOC]
        m = {
            "xbf": xp.astype(BF), "xt": np.ascontiguousarray(xp.T).astype(BF),
            "ncs": ncs, "idx": idx_arrs[c], "slots": slot_arrs[c],
            "ptab": pt_arrs[c],
            "w1e": w1e, "w2e": w2e, "iotax": iotax,
            "ident": identm,
        }
        if not ln_trivial:
            m["badd"] = badd
        if not b1_zero:
            m["b1r"] = np.tile(b1[perm1][None, :], (128, 1)).astype(np.float32)
        if not b2_zero:
            m["b2r"] = np.tile(b2[None, :], (128, 1)).astype(np.float32)
        in_maps.append(m)
    meta = (NW, Woff, SWoff, Wtot, SWtot, nexact, ln_trivial, b1_zero, b2_zero)
    return meta, in_maps


def kernel(**inputs):
    _install_ntff_hook()
    from concourse.bass_utils import run_bass_kernel_spmd

    meta, in_maps = _host_prep(inputs)
    NW, Woff, SWoff, Wtot, SWtot, nexact, ln_trivial, b1_zero, b2_zero = meta
    key = (Wtot, SWtot, ln_trivial, b1_zero, b2_zero, NW.tobytes(),
           nexact.tobytes())
    if key not in _cache:
        _cache[key] = _build(NW, Woff, SWoff, Wtot, SWtot, nexact,
                             ln_trivial, b1_zero, b2_zero)
    nc = _cache[key]

    trace = bool(int(__import__("os").environ.get("KERNEL_TRACE", "0")))
    res = run_bass_kernel_spmd(nc, in_maps, core_ids=list(range(NCORE)),
                               trace=trace)
    kernel.last_exec_time_ns = res.exec_time_ns
    out = np.concatenate([res.results[c]["out"][:NLOC] for c in range(NCORE)], 0)
    return out.astype(np.float32)


kernel.last_exec_time_ns = None


# revision 8
# speedup vs baseline: 1.5227x; 1.0206x over previous
"""Distributed 2-layer GAT (nn_AlignHead) on 8 TRN2 NeuronCores.

Strategy: shard nodes (dst) contiguously across 8 cores. Per core:
  Phase A: LayerNorm + h@W1_ext dense matmul -> per-node table rows
           [g1 (c-major, 512) | a_src1 (8) | a_dst1 (8) | pad] bf16.
           The table is AllGathered in TWO halves (lo = global rows
           [0,31744) = every core's local rows [0,3968); hi = the rest),
           each its own Shared tensor (int16 gather indices + exact deps).
  Phase B: per dst-tile (128 dsts): dma_gather edge src rows (runs:
           lo-half, hi-half, self-loops via dense local DMA), segment
           softmax via indicator matmuls.  Software-pipelined three deep:
           lo-gather emission runs SKEW tiles ahead (so lo traffic flows
           while the hi AllGather lands — the GpSimd queue is in-order);
           P/P^T/zb (a_dst broadcast) run one tile ahead of the main
           compute so the PE never head-of-line blocks on the DVE softmax
           chain (which would let the PE HAM throttle back to 1.2 GHz).
           The weighted-aggregation matmuls are chunked 4 windows at a
           time, interleaved with the DVE stg builds.  SELF window skips
           the zb matmul (z_self = a_src_row + a_dst tile, pure DVE).
           ELU -> h2, PE transpose, dense h2@W2_ext -> table2; two-half
           AllGather as in phase A.
  Phase C: conv2 edge phase (1 head): attention scalar folded INTO P
           (per-partition scale), single 258-col matmul per window with a
           ones-column denominator; a_dst2 broadcast reuses the SAME
           host-built P^T stream (no PE transposes); same pipelining.

Self-contained: hardcodes the problem shapes; compiles on first call.
"""
import sys
import types

import numpy as np
import ml_dtypes

# ---------------------------------------------------------------- constants
NCORE = 8
N = 50000
E = 500000
D = 256
H1, C1 = 8, 64
DH = 512            # H1*C1
NEG = 0.2
EPS = 1e-5
NLOC = 6250         # nodes per core
NPAD = 6272         # 49*128
T = 49              # dst tiles per core
LOT = 31            # tiles in the lo half (31*128*8 = 31744 < 32768: int16)
LOROWS = LOT * 128          # 3968 local rows in lo half
HIROWS = NPAD - LOROWS      # 2304 local rows in hi half
ELEM1 = 640         # bf16 elems per conv1 table row (1280 B)
ELEM2 = 384         # bf16 elems per conv2 table row (768 B)
NQ = 4              # swdge queues
BF = ml_dtypes.bfloat16
GCAP = 4            # max windows per dma_gather call
SKEW = 3            # lo-gather emission runs SKEW tiles ahead of compute
SKEWC = 6           # deeper skew in phase C (SBUF is lighter there)
SUBT = [0, 16, 31, 40, 49]   # AG sub-chunk tile boundaries
SUB1 = 16 * 128              # 2048 local rows in lo sub-chunk 0
SUB2 = 9 * 128               # 1152 local rows in hi sub-chunk 0
WCH = 4             # aggregation window-chunk size

_cache = {}


def _install_ntff_hook():
    if "antenv.axon_hooks" in sys.modules:
        return
    try:
        import antenv
        mod = types.ModuleType("antenv.axon_hooks")
        _h = [None]
        mod.set_axon_ntff_profile_hook = lambda h: _h.__setitem__(0, h)
        mod.get_axon_ntff_profile_hook = lambda: _h[0]
        sys.modules["antenv.axon_hooks"] = mod
        antenv.axon_hooks = mod
        from trn_agent_boot.trn_boot import _ntff_profile_via_ctypes
        mod.set_axon_ntff_profile_hook(
            _ntff_profile_via_ctypes("/opt/axon/libaxon_pjrt.so"))
    except Exception:
        pass


def _prep_edges(edge_index):
    """Partition + window-pad edges. Runs per tile: 0=lo half, 1=hi half,
    2=self-loops (dense DMA, no gather)."""
    src = np.asarray(edge_index[0]).astype(np.int64)
    dst = np.asarray(edge_index[1]).astype(np.int64)
    loops = np.arange(N, dtype=np.int64)
    src = np.concatenate([src, loops])
    dst = np.concatenate([dst, loops])
    is_self = np.zeros(len(src), bool)
    is_self[E:] = True

    core = dst // NLOC
    ldst = dst % NLOC
    tilei = ldst // 128
    slot = ldst % 128
    s_core = src // NLOC
    s_loc = src % NLOC
    in_lo = s_loc < LOROWS
    srow = np.where(in_lo, s_core * LOROWS + s_loc,
                    s_core * HIROWS + (s_loc - LOROWS))
    run = np.where(is_self, 2, np.where(in_lo, 0, 1))

    nrun = 3
    cnt = np.zeros((NCORE, T, nrun), np.int64)
    np.add.at(cnt, (core, tilei, run), 1)
    NW = np.maximum(1, np.ceil(cnt.max(axis=0) / 128).astype(np.int64))  # [T,nrun]
    nexact = cnt.max(axis=0)

    Woff = np.zeros((T, nrun), np.int64)
    w = 0
    for t in range(T):
        for r in range(nrun):
            Woff[t, r] = w
            w += NW[t, r]
    Wtot = int(w)

    SWoff = np.zeros((T, nrun), np.int64)
    sw = 0
    for t in range(T):
        sw += sw & 1
        for r in range(nrun):
            SWoff[t, r] = sw
            sw += NW[t, r]
    SWtot = int(sw + (sw & 1))

    order = np.lexsort((ldst, run, tilei, core))
    src_s = srow[order]
    core_s = core[order]
    tile_s = tilei[order]
    run_s = run[order]
    slot_s = slot[order]

    idx_arrs, slot_arrs, pt_arrs = [], [], []
    seg_key = ((core_s * T + tile_s) * nrun + run_s)
    bounds = np.searchsorted(seg_key, np.arange(NCORE * T * nrun + 1))
    srange = np.arange(128)
    NWT1 = int(NW.sum(axis=1).max())
    for c in range(NCORE):
        idx16 = np.zeros((16, Wtot * 8), np.int16)
        slots = np.full((128, SWtot), 128.0, np.float32)
        startv = np.zeros((128, SWtot), np.float32)
        endv = np.zeros((128, SWtot), np.float32)
        for t in range(T):
            for r in range(nrun):
                k = (c * T + t) * nrun + r
                a, b = bounds[k], bounds[k + 1]
                n = b - a
                nw = int(NW[t, r])
                assert n <= nw * 128
                rows = src_s[a:b]
                sl = slot_s[a:b]
                j = np.arange(n)
                w0 = int(Woff[t, r])
                idx16[j % 16, w0 * 8 + j // 16] = rows.astype(np.int16)
                s0 = int(SWoff[t, r])
                slots[j % 128, s0 + j // 128] = sl
                for w in range(nw):
                    wsl = sl[w * 128:(w + 1) * 128]
                    startv[:, s0 + w] = np.searchsorted(wsl, srange, "left")
                    endv[:, s0 + w] = np.searchsorted(wsl, srange, "right")
        idx_arrs.append(np.tile(idx16, (8, 1)))
        slot_arrs.append(slots.astype(BF))
        ptab = np.zeros((T * 128, NWT1 * 128), BF)
        jj = np.arange(128)
        for t in range(T):
            s0 = int(SWoff[t, 0])
            nwt = int(NW[t].sum())
            sv = startv[:, s0:s0 + nwt]
            ev = endv[:, s0:s0 + nwt]
            exp = ((jj[None, None, :] >= sv[:, :, None])
                   & (jj[None, None, :] < ev[:, :, None]))
            ptab[t * 128:(t + 1) * 128, 0:nwt * 128] = \
                exp.reshape(128, nwt * 128).astype(BF)
        pt_arrs.append(ptab)
    return (NW, Woff, SWoff, Wtot, SWtot, nexact,
            idx_arrs, slot_arrs, pt_arrs)


def _build(NW, Woff, SWoff, Wtot, SWtot, nexact, ln_trivial, b1_zero, b2_zero):
    import concourse.bacc as bacc
    import concourse.mybir as mybir
    import concourse.tile as tile

    f32 = mybir.dt.float32
    bf = mybir.dt.bfloat16
    i16 = mybir.dt.int16
    AF = mybir.ActivationFunctionType
    ALU = mybir.AluOpType
    NWT1 = int(NW.sum(axis=1).max())

    nc = bacc.Bacc("TRN2", target_bir_lowering=False, debug=False,
                   num_devices=NCORE, num_swdge_queues=NQ)

    xb_in = nc.declare_dram_parameter("xbf", [NPAD, D], bf, isOutput=False)
    xt_in = nc.declare_dram_parameter("xt", [D, NPAD], bf, isOutput=False)
    ncs_in = nc.declare_dram_parameter("ncs", [128, DH + 16], f32, isOutput=False)
    idx_in = nc.declare_dram_parameter("idx", [128, Wtot * 8], i16, isOutput=False)
    sl_in = nc.declare_dram_parameter("slots", [128, SWtot], bf, isOutput=False)
    pt_in = nc.declare_dram_parameter("ptab", [T * 128, NWT1 * 128], bf,
                                      isOutput=False)
    w1_in = nc.declare_dram_parameter("w1e", [D, DH + 16], bf, isOutput=False)
    w2_in = nc.declare_dram_parameter("w2e", [DH, D + 2], bf, isOutput=False)
    io_in = nc.declare_dram_parameter("iotax", [128, 128 * NWT1], bf, isOutput=False)
    id_in = nc.declare_dram_parameter("ident", [128, 128], bf, isOutput=False)
    badd_in = b1_in = b2_in = None
    if not ln_trivial:
        badd_in = nc.declare_dram_parameter("badd", [128, DH + 16], f32,
                                            isOutput=False)
    if not b1_zero:
        b1_in = nc.declare_dram_parameter("b1r", [128, DH], f32, isOutput=False)
    if not b2_zero:
        b2_in = nc.declare_dram_parameter("b2r", [128, D], f32, isOutput=False)
    out_ext = nc.declare_dram_parameter("out", [NPAD, D], f32, isOutput=True)

    HALFR = [LOROWS, HIROWS]
    HALFT = [0, LOT, T]
    tab1_locs = [nc.dram_tensor(f"tab1_loc{q}", [HALFR[q], ELEM1], bf)
                 for q in range(2)]
    tab2_locs = [nc.dram_tensor(f"tab2_loc{q}", [HALFR[q], ELEM2], bf)
                 for q in range(2)]

    qrot = [0]

    def nextq():
        q = qrot[0]
        qrot[0] = (q + 1) % NQ
        return q

    with tile.TileContext(nc) as tc:
        with (
            tc.tile_pool(name="const", bufs=1) as cpool,
            tc.tile_pool(name="dram", bufs=1, space="DRAM") as dpool,
        ):
            tab1_fulls = [dpool.tile([NCORE * HALFR[q], ELEM1], bf,
                                     addr_space="Shared", name=f"t1f{q}")
                          for q in range(2)]
            tab2_fulls = [dpool.tile([NCORE * HALFR[q], ELEM2], bf,
                                     addr_space="Shared", name=f"t2f{q}")
                          for q in range(2)]

            # ---- constants to SBUF
            w1e = cpool.tile([128, 2, DH + 16], bf)
            nc.sync.dma_start(w1e[:], w1_in[:].rearrange("(k p) f -> p k f", p=128))
            w2e = cpool.tile([128, 4, D + 2], bf)
            nc.sync.dma_start(w2e[:], w2_in[:].rearrange("(k p) f -> p k f", p=128))
            iotax = cpool.tile([128, 128 * NWT1], bf)
            nc.sync.dma_start(iotax[:], io_in[:])
            ncs_sb = cpool.tile([128, DH + 16], f32)
            nc.sync.dma_start(ncs_sb[:], ncs_in[:])
            ident = cpool.tile([128, 128], bf)
            nc.sync.dma_start(ident[:], id_in[:])
            slots_sb = cpool.tile([128, SWtot], bf)
            nc.sync.dma_start(slots_sb[:], sl_in[:])
            idx_sb = cpool.tile([128, Wtot * 8], i16)
            nc.sync.dma_start(idx_sb[:], idx_in[:])
            adst1 = cpool.tile([128, T * 8], bf)
            adst2 = cpool.tile([128, T], bf)
            if not ln_trivial:
                badd_sb = cpool.tile([128, DH + 16], f32)
                nc.sync.dma_start(badd_sb[:], badd_in[:])
            if not b1_zero:
                b1_sb = cpool.tile([128, DH], f32)
                nc.sync.dma_start(b1_sb[:], b1_in[:])
            if not b2_zero:
                b2_sb = cpool.tile([128, D], f32)
                nc.sync.dma_start(b2_sb[:], b2_in[:])

            iotax3 = iotax[:].rearrange("p (s w) -> p s w", w=NWT1)

            # ====== PHASE A ======
            pha = tc.tile_pool(name="phA", bufs=4)
            iop = pha.__enter__()
            wk_cm = tc.tile_pool(name="wkA", bufs=4)
            wkp = wk_cm.__enter__()
            sm_cm = tc.tile_pool(name="smA", bufs=4)
            smp = sm_cm.__enter__()
            psA_cm = tc.tile_pool(name="psA", bufs=4, space="PSUM")
            psA = psA_cm.__enter__()
            for t in range(T):
                xb = iop.tile([128, D], bf, tag="xb")
                nc.sync.dma_start(xb[:], xb_in[t * 128:(t + 1) * 128, :])
                xT = iop.tile([128, 2, 128], bf, tag="xT")
                nc.sync.dma_start(
                    xT[:], xt_in[:, t * 128:(t + 1) * 128]
                    .rearrange("(k p) n -> p k n", p=128))
                s1 = smp.tile([128, 1], f32, tag="s1")
                nc.vector.reduce_sum(s1[:], xb[:], axis=mybir.AxisListType.X)
                mu = smp.tile([128, 1], f32, tag="mu")
                nc.vector.tensor_scalar_mul(mu[:], s1[:], 1.0 / D)
                sqj = wkp.tile([128, D], f32, tag="sqj")
                s2 = smp.tile([128, 1], f32, tag="s2")
                nc.scalar.activation(sqj[:], xb[:], AF.Square, accum_out=s2[:])
                v1 = smp.tile([128, 1], f32, tag="v1")
                nc.vector.tensor_scalar(v1[:], s2[:], 1.0 / D, EPS, ALU.mult, ALU.add)
                mu2 = smp.tile([128, 1], f32, tag="mu2")
                nc.vector.tensor_mul(mu2[:], mu[:], mu[:])
                var = smp.tile([128, 1], f32, tag="var")
                nc.vector.tensor_tensor(var[:], v1[:], mu2[:], ALU.subtract)
                sd = smp.tile([128, 1], f32, tag="sd")
                nc.scalar.activation(sd[:], var[:], AF.Sqrt)
                rstd = smp.tile([128, 1], f32, tag="rstd")
                nc.vector.reciprocal(rstd[:], sd[:])
                kap = smp.tile([128, 1], f32, tag="kap")
                nc.vector.tensor_mul(kap[:], mu[:], rstd[:])
                ps1 = psA.tile([128, DH], f32, tag="ps1")
                ps1b = psA.tile([128, 16], f32, tag="ps1b")
                for k in range(2):
                    nc.tensor.matmul(ps1[:], xT[:, k, :], w1e[:, k, 0:DH],
                                     start=(k == 0), stop=(k == 1))
                    nc.tensor.matmul(ps1b[:], xT[:, k, :],
                                     w1e[:, k, DH:DH + 16],
                                     start=(k == 0), stop=(k == 1))
                t1 = wkp.tile([128, DH + 16], bf, tag="t1")
                nc.scalar.activation(t1[:, 0:DH], ps1[:], AF.Copy, scale=rstd[:])
                nc.scalar.activation(t1[:, DH:DH + 16], ps1b[:], AF.Copy,
                                     scale=rstd[:])
                tbx = iop.tile([128, DH + 16], bf, tag="tb1")
                nc.vector.scalar_tensor_tensor(
                    tbx[:, 0:DH + 16], ncs_sb[:], kap[:], t1[:],
                    ALU.mult, ALU.add)
                if not ln_trivial:
                    nc.vector.tensor_add(tbx[:, 0:DH + 16], tbx[:, 0:DH + 16],
                                         badd_sb[:])
                nc.scalar.copy(adst1[:, t * 8:(t + 1) * 8],
                               tbx[:, DH + 8:DH + 16])
                qch = 0 if t < LOT else 1
                r0 = t * 128 - (0 if t < LOT else LOROWS)
                nc.sync.dma_start(tab1_locs[qch][r0:r0 + 128, 0:DH + 8],
                                  tbx[:, 0:DH + 8])
                if t == HALFT[qch + 1] - 1:
                    nc.gpsimd.collective_compute(
                        "AllGather", mybir.AluOpType.bypass,
                        replica_groups=[list(range(NCORE))],
                        ins=[tab1_locs[qch][:]],
                        outs=[tab1_fulls[qch].opt()],
                    )

            psA_cm.__exit__(None, None, None)
            sm_cm.__exit__(None, None, None)
            wk_cm.__exit__(None, None, None)
            pha.__exit__(None, None, None)

            # ================= PHASE B: conv1 edges + dense2 =================
            # 3-deep software pipeline:
            #   stage 0 (tile tt):      lo-gather emission
            #   stage 1 (tile t1=tt-SKEW+1): hi-gather, self DMA, Pt stream,
            #                                P build, zb matmuls
            #   stage 2 (tile t2=tt-SKEW):   softmax + chunked aggregation +
            #                                dense2 + table write + AG
            phb = tc.tile_pool(name="phB", bufs=4)
            iop = phb.__enter__()
            wk_cm = tc.tile_pool(name="wkB", bufs=3)
            wkp = wk_cm.__enter__()
            sm_cm = tc.tile_pool(name="smB", bufs=4)
            smp = sm_cm.__enter__()
            ga_cm = tc.tile_pool(name="gaB", bufs=SKEW + 3)
            gap = ga_cm.__enter__()
            st_cm = tc.tile_pool(name="stB", bufs=2)
            stp = st_cm.__enter__()
            psZ_cm = tc.tile_pool(name="psZ", bufs=3, space="PSUM")
            psZ = psZ_cm.__enter__()
            psD_cm = tc.tile_pool(name="psD", bufs=1, space="PSUM")
            psD = psD_cm.__enter__()
            psC_cm = tc.tile_pool(name="psC", bufs=2, space="PSUM")
            psC = psC_cm.__enter__()
            gts, Ps, Pts, zbs = {}, {}, {}, {}
            for tt in range(T + SKEW):
                if tt < T:
                    gt = gap.tile([128, NWT1, ELEM1], bf, tag="gt1")
                    gts[tt] = gt
                    for (rbase, w0g, w0l, nw, nidx) in _calls(
                            tt, 0, 0, Woff, NW, nexact):
                        nc.gpsimd.dma_gather(
                            gt[:, w0l:w0l + nw, :], tab1_fulls[0][:],
                            idx_sb[:, w0g * 8:(w0g + nw) * 8],
                            num_idxs=nidx, num_idxs_reg=nidx,
                            elem_size=ELEM1, queue_num=nextq(),
                        )
                t1 = tt - SKEW + 1
                if 0 <= t1 < T:
                    gt = gts[t1]
                    nws = [int(NW[t1, r]) for r in range(3)]
                    nwt1 = sum(nws)
                    ws1 = nwt1 - 1
                    for (rbase, w0g, w0l, nw, nidx) in _calls(
                            t1, 1, nws[0], Woff, NW, nexact):
                        nc.gpsimd.dma_gather(
                            gt[:, w0l:w0l + nw, :], tab1_fulls[1][:],
                            idx_sb[:, w0g * 8:(w0g + nw) * 8],
                            num_idxs=nidx, num_idxs_reg=nidx,
                            elem_size=ELEM1, queue_num=nextq(),
                        )
                    qch = 0 if t1 < LOT else 1
                    r0s = t1 * 128 - (0 if t1 < LOT else LOROWS)
                    nc.sync.dma_start(gt[:, ws1, 0:DH + 8],
                                      tab1_locs[qch][r0s:r0s + 128, 0:DH + 8])
                    S0 = int(SWoff[t1, 0])
                    P = stp.tile([128, 128 * NWT1], bf, tag="P1")
                    Ps[t1] = P
                    Pv = P[:, :128 * nwt1].rearrange("p (s w) -> p s w", w=nwt1)
                    nc.vector.tensor_tensor(
                        Pv[:, :, :],
                        slots_sb[:, S0:S0 + nwt1].unsqueeze(1)
                        .broadcast_to([128, 128, nwt1]),
                        iotax3[:, :, 0:nwt1], ALU.is_equal)
                    Pt = stp.tile([128, NWT1, 128], bf, tag="Pt1")
                    Pts[t1] = Pt
                    nc.sync.dma_start(
                        Pt[:], pt_in[t1 * 128:(t1 + 1) * 128, :]
                        .rearrange("p (w j) -> p w j", j=128))
                    zb = psZ.tile([128, NWT1 * 8], f32, tag="zbt")
                    zbs[t1] = zb
                    for w in range(ws1):
                        nc.tensor.matmul(zb[:, w * 8:(w + 1) * 8],
                                         Pt[:, w, :],
                                         adst1[:, t1 * 8:(t1 + 1) * 8],
                                         start=True, stop=True)
                if tt < SKEW:
                    continue
                t = tt - SKEW
                gt = gts.pop(t)
                P = Ps.pop(t)
                Pt = Pts.pop(t)
                zb = zbs.pop(t)
                nws = [int(NW[t, r]) for r in range(3)]
                nwt = sum(nws)
                ws = nwt - 1
                Pv = P[:, :128 * nwt].rearrange("p (s w) -> p s w", w=nwt)
                z = smp.tile([128, NWT1 * 8], f32, tag="z1")
                nc.vector.scalar_tensor_tensor(
                    z[:, :ws * 8].rearrange("p (w d) -> p w d", d=8),
                    zb[:, :ws * 8].rearrange("p (w d) -> p w d", d=8), 1.0,
                    gt[:, 0:ws, DH:DH + 8],
                    ALU.mult, ALU.add)
                nc.vector.tensor_tensor(
                    z[:, ws * 8:nwt * 8], gt[:, ws, DH:DH + 8],
                    adst1[:, t * 8:(t + 1) * 8], ALU.add)
                e2 = smp.tile([128, NWT1 * 8], f32, tag="e21")
                nc.scalar.activation(e2[:, :nwt * 8], z[:, :nwt * 8], AF.Exp, scale=NEG)
                e1 = smp.tile([128, NWT1 * 8], f32, tag="e11")
                nc.scalar.activation(e1[:, :nwt * 8], z[:, :nwt * 8], AF.Exp)
                stg = stp.tile([128, NWT1, 8 + DH], bf, tag="stg1")
                oc = psC.tile([128, 1024], f32, tag="oc1")
                # chunked: DVE builds p/W'' for WCH windows, PE aggregates
                # them while DVE builds the next chunk
                for c0 in range(0, nwt, WCH):
                    c1 = min(c0 + WCH, nwt)
                    nc.vector.tensor_tensor(
                        stg[:, c0:c1, 0:8],
                        e1[:, c0 * 8:c1 * 8].rearrange("p (w d) -> p w d", d=8),
                        e2[:, c0 * 8:c1 * 8].rearrange("p (w d) -> p w d", d=8),
                        ALU.max)
                    nc.vector.tensor_mul(
                        stg[:, c0:c1, 8:8 + DH].rearrange(
                            "p w (c h) -> p w c h", h=8),
                        gt[:, c0:c1, 0:DH].rearrange("p w (c h) -> p w c h", h=8),
                        stg[:, c0:c1, 0:8].unsqueeze(2)
                        .broadcast_to([128, c1 - c0, 64, 8]))
                    for w in range(c0, c1):
                        nc.tensor.matmul(oc[:, 0:8], Pv[:, :, w], stg[:, w, 0:8],
                                         start=(w == 0), stop=(w == nwt - 1))
                        nc.tensor.matmul(oc[:, 512:512 + DH], Pv[:, :, w],
                                         stg[:, w, 8:8 + DH],
                                         start=(w == 0), stop=(w == nwt - 1))
                den = smp.tile([128, 8], f32, tag="den1")
                nc.vector.tensor_scalar_max(den[:], oc[:, 0:8], 1e-30)
                rec = smp.tile([128, 8], f32, tag="rec1")
                nc.vector.reciprocal(rec[:], den[:])
                o1 = wkp.tile([128, DH], bf, tag="o1")
                nc.vector.tensor_tensor(
                    o1[:].rearrange("p (c h) -> p c h", h=8),
                    oc[:, 512:512 + DH].rearrange("p (c h) -> p c h", h=8),
                    rec[:].unsqueeze(1).broadcast_to([128, 64, 8]),
                    ALU.mult)
                if not b1_zero:
                    o1f = wkp.tile([128, DH], f32, tag="o1f")
                    nc.vector.tensor_add(o1f[:], o1[:], b1_sb[:])
                    o1 = o1f
                pos = wkp.tile([128, DH], bf, tag="pos")
                nc.scalar.activation(pos[:], o1[:], AF.Relu)
                rneg = wkp.tile([128, DH], bf, tag="rneg")
                nc.scalar.activation(rneg[:], o1[:], AF.Relu, scale=-1.0)
                en = wkp.tile([128, DH], bf, tag="en")
                nc.scalar.activation(en[:], rneg[:], AF.Exp, scale=-1.0)
                h2 = wkp.tile([128, DH], bf, tag="h2")
                nc.vector.scalar_tensor_tensor(h2[:], pos[:], -1.0, en[:],
                                               ALU.add, ALU.add)
                hT2 = wkp.tile([128, 4, 128], bf, tag="hT2")
                pst = psZ.tile([128, 4, 128], bf, tag="zbt")
                for k in range(4):
                    nc.tensor.transpose(pst[:, k, :], h2[:, k * 128:(k + 1) * 128], ident[:])
                nc.scalar.copy(hT2[:], pst[:])
                ps2 = psD.tile([128, D + 2], f32, tag="ps2")
                for k in range(4):
                    nc.tensor.matmul(ps2[:], hT2[:, k, :], w2e[:, k, :],
                                     start=(k == 0), stop=(k == 3))
                nc.scalar.copy(adst2[:, t:t + 1], ps2[:, D + 1:D + 2])
                tb2 = iop.tile([128, ELEM2], bf, tag="tb2")
                nc.scalar.copy(tb2[:, 0:D + 1], ps2[:, 0:D + 1])
                nc.vector.memset(tb2[:, D + 1:D + 2], 1.0)
                qch = 0 if t < LOT else 1
                r0 = t * 128 - (0 if t < LOT else LOROWS)
                nc.sync.dma_start(tab2_locs[qch][r0:r0 + 128, 0:D + 2],
                                  tb2[:, 0:D + 2])
                if t == HALFT[qch + 1] - 1:
                    nc.gpsimd.collective_compute(
                        "AllGather", mybir.AluOpType.bypass,
                        replica_groups=[list(range(NCORE))],
                        ins=[tab2_locs[qch][:]],
                        outs=[tab2_fulls[qch].opt()],
                    )

            psC_cm.__exit__(None, None, None)
            psD_cm.__exit__(None, None, None)
            psZ_cm.__exit__(None, None, None)
            st_cm.__exit__(None, None, None)
            ga_cm.__exit__(None, None, None)
            sm_cm.__exit__(None, None, None)
            wk_cm.__exit__(None, None, None)
            phb.__exit__(None, None, None)

            # ================= PHASE C: conv2 edges =================
            phc = tc.tile_pool(name="phC", bufs=3)
            iop = phc.__enter__()
            sm_cm = tc.tile_pool(name="smC", bufs=3)
            smp = sm_cm.__enter__()
            ga_cm = tc.tile_pool(name="gaC", bufs=SKEWC + 3)
            gap = ga_cm.__enter__()
            st_cm = tc.tile_pool(name="stC", bufs=2)
            stp = st_cm.__enter__()
            psZ_cm = tc.tile_pool(name="psZC", bufs=2, space="PSUM")
            psZ = psZ_cm.__enter__()
            psC_cm = tc.tile_pool(name="psCC", bufs=2, space="PSUM")
            psC = psC_cm.__enter__()
            gts, Ps, Pts, zbs = {}, {}, {}, {}
            for tt in range(T + SKEWC):
                if tt < T:
                    gt = gap.tile([128, NWT1, ELEM2], bf, tag="gt2")
                    gts[tt] = gt
                    for (rbase, w0g, w0l, nw, nidx) in _calls(
                            tt, 0, 0, Woff, NW, nexact):
                        nc.gpsimd.dma_gather(
                            gt[:, w0l:w0l + nw, :], tab2_fulls[0][:],
                            idx_sb[:, w0g * 8:(w0g + nw) * 8],
                            num_idxs=nidx, num_idxs_reg=nidx,
                            elem_size=ELEM2, queue_num=nextq(),
                        )
                t1 = tt - SKEWC + 1
                if 0 <= t1 < T:
                    gt = gts[t1]
                    nws = [int(NW[t1, r]) for r in range(3)]
                    nwt1 = sum(nws)
                    ws1 = nwt1 - 1
                    for (rbase, w0g, w0l, nw, nidx) in _calls(
                            t1, 1, nws[0], Woff, NW, nexact):
                        nc.gpsimd.dma_gather(
                            gt[:, w0l:w0l + nw, :], tab2_fulls[1][:],
                            idx_sb[:, w0g * 8:(w0g + nw) * 8],
                            num_idxs=nidx, num_idxs_reg=nidx,
                            elem_size=ELEM2, queue_num=nextq(),
                        )
                    qch = 0 if t1 < LOT else 1
                    r0s = t1 * 128 - (0 if t1 < LOT else LOROWS)
                    nc.sync.dma_start(gt[:, ws1, 0:D + 2],
                                      tab2_locs[qch][r0s:r0s + 128, 0:D + 2])
                    S0 = int(SWoff[t1, 0])
                    P = stp.tile([128, 128 * NWT1], bf, tag="P2")
                    Ps[t1] = P
                    Pv = P[:, :128 * nwt1].rearrange("p (s w) -> p s w", w=nwt1)
                    nc.vector.tensor_tensor(
                        Pv[:, :, :],
                        slots_sb[:, S0:S0 + nwt1].unsqueeze(1)
                        .broadcast_to([128, 128, nwt1]),
                        iotax3[:, :, 0:nwt1], ALU.is_equal)
                    Pt = stp.tile([128, NWT1, 128], bf, tag="Pt2")
                    Pts[t1] = Pt
                    nc.sync.dma_start(
                        Pt[:], pt_in[t1 * 128:(t1 + 1) * 128, :]
                        .rearrange("p (w j) -> p w j", j=128))
                    zb = psZ.tile([128, NWT1], f32, tag="zb2")
                    zbs[t1] = zb
                    for w in range(ws1):
                        nc.tensor.matmul(zb[:, w:w + 1],
                                         Pt[:, w, :],
                                         adst2[:, t1:t1 + 1],
                                         start=True, stop=True)
                if tt < SKEWC:
                    continue
                t = tt - SKEWC
                gt = gts.pop(t)
                P = Ps.pop(t)
                Pt = Pts.pop(t)
                zb = zbs.pop(t)
                nws = [int(NW[t, r]) for r in range(3)]
                nwt = sum(nws)
                ws = nwt - 1
                Pv = P[:, :128 * nwt].rearrange("p (s w) -> p s w", w=nwt)
                z = smp.tile([128, NWT1], f32, tag="z2")
                nc.vector.scalar_tensor_tensor(
                    z[:, :ws].rearrange("p (w d) -> p w d", d=1),
                    zb[:, :ws].rearrange("p (w d) -> p w d", d=1), 1.0,
                    gt[:, 0:ws, D:D + 1],
                    ALU.mult, ALU.add)
                nc.vector.tensor_tensor(
                    z[:, ws:nwt], gt[:, ws, D:D + 1],
                    adst2[:, t:t + 1], ALU.add)
                e2 = smp.tile([128, NWT1], f32, tag="e22")
                nc.scalar.activation(e2[:, :nwt], z[:, :nwt], AF.Exp, scale=NEG)
                e1 = smp.tile([128, NWT1], f32, tag="e12")
                nc.scalar.activation(e1[:, :nwt], z[:, :nwt], AF.Exp)
                p2 = smp.tile([128, NWT1], bf, tag="p2")
                nc.vector.tensor_max(p2[:, :nwt], e1[:, :nwt], e2[:, :nwt])
                oc2 = psC.tile([128, D + 2], f32, tag="oc2")
                for c0 in range(0, nwt, WCH):
                    c1 = min(c0 + WCH, nwt)
                    nc.vector.tensor_tensor(
                        Pv[:, :, c0:c1], Pv[:, :, c0:c1],
                        p2[:, c0:c1].unsqueeze(1)
                        .broadcast_to([128, 128, c1 - c0]),
                        ALU.mult)
                    for w in range(c0, c1):
                        nc.tensor.matmul(oc2[:], Pv[:, :, w], gt[:, w, 0:D + 2],
                                         start=(w == 0), stop=(w == nwt - 1))
                den = smp.tile([128, 1], f32, tag="den2")
                nc.vector.tensor_scalar_max(den[:], oc2[:, D + 1:D + 2], 1e-30)
                rec = smp.tile([128, 1], f32, tag="rec2")
                nc.vector.reciprocal(rec[:], den[:])
                outt = iop.tile([128, D], f32, tag="outt")
                nc.vector.tensor_scalar(outt[:], oc2[:, 0:D], rec[:], None, ALU.mult)
                if not b2_zero:
                    nc.vector.tensor_add(outt[:], outt[:], b2_sb[:])
                nc.sync.dma_start(out_ext[t * 128:(t + 1) * 128, :], outt[:])
            psC_cm.__exit__(None, None, None)
            psZ_cm.__exit__(None, None, None)
            st_cm.__exit__(None, None, None)
            ga_cm.__exit__(None, None, None)
            sm_cm.__exit__(None, None, None)
            phc.__exit__(None, None, None)

    nc.compile()
    return nc


def _subrng(si):
    """(local_row_start, local_row_end, global_row_start) of AG sub-chunk si
    within its half tensor."""
    if si == 0:
        return 0, SUB1, 0
    if si == 1:
        return SUB1, LOROWS, NCORE * SUB1
    if si == 2:
        return 0, SUB2, 0
    return SUB2, HIROWS, NCORE * SUB2


def _calls(t, r, lbase, Woff, NW, nexact):
    """Gather call plan for run r of tile t:
    (run, global_w0, local_w0, nw, nidx)."""
    out = []
    nwr = int(NW[t, r])
    w0 = int(Woff[t, r])
    ntot = int(nexact[t, r]) if t >= 2 * (SKEWC + 1) else nwr * 128
    done = 0
    while done < nwr:
        nw = min(GCAP, nwr - done)
        nidx = min(nw * 128, max(1, ntot - done * 128))
        out.append((r, w0 + done, lbase + done, nw, nidx))
        done += nw
    return out


def _host_prep(inputs):
    edge_index = np.asarray(inputs["edge_index"])
    x = np.asarray(inputs["x"], np.float32)
    ln_w = np.asarray(inputs["ln_w"], np.float32)
    ln_b = np.asarray(inputs["ln_b"], np.float32)
    W1 = np.asarray(inputs["W1"], np.float32)
    a_s1 = np.asarray(inputs["att_src1"], np.float32)
    a_d1 = np.asarray(inputs["att_dst1"], np.float32)
    b1 = np.asarray(inputs["b1"], np.float32)
    W2 = np.asarray(inputs["W2"], np.float32)
    a_s2 = np.asarray(inputs["att_src2"], np.float32)
    a_d2 = np.asarray(inputs["att_dst2"], np.float32)
    b2 = np.asarray(inputs["b2"], np.float32)

    (NW, Woff, SWoff, Wtot, SWtot, nexact, idx_arrs, slot_arrs,
     pt_arrs) = _prep_edges(edge_index)
    NWT1 = int(NW.sum(axis=1).max())

    ln_trivial = bool(np.all(ln_w == 1.0) and np.all(ln_b == 0.0))

    perm1 = np.empty(DH, np.int64)
    for h in range(H1):
        for c in range(C1):
            perm1[c * 8 + h] = h * C1 + c
    wsrc1 = np.stack([W1[:, h * C1:(h + 1) * C1] @ a_s1[h] for h in range(H1)], 1)
    wdst1 = np.stack([W1[:, h * C1:(h + 1) * C1] @ a_d1[h] for h in range(H1)], 1)
    w1o = np.concatenate([W1[:, perm1], wsrc1, wdst1], axis=1)    # [256, 528]
    w1f = ln_w[:, None] * w1o
    w1e = w1f.astype(BF)
    ncs = np.tile((-w1f.sum(axis=0))[None, :], (128, 1)).astype(np.float32)
    badd = np.tile((ln_b @ w1o)[None, :], (128, 1)).astype(np.float32)

    W2r = W2[perm1, :]
    wsrc2 = W2r @ a_s2[0]
    wdst2 = W2r @ a_d2[0]
    w2e = np.concatenate([W2r, wsrc2[:, None], wdst2[:, None]], axis=1).astype(BF)

    iotax = np.zeros((128, 128 * NWT1), np.float32)
    for s in range(128):
        iotax[:, s * NWT1:(s + 1) * NWT1] = s
    iotax = iotax.astype(BF)
    identm = np.eye(128).astype(BF)

    b1_zero = bool(np.all(b1 == 0.0))
    b2_zero = bool(np.all(b2 == 0.0))

    in_maps = []
    for c in range(NCORE):
        xp = np.zeros((NPAD, D), np.float32)
        xp[:NLOC] = x[c * NLOC:(c + 1) * NLOC]
        m = {
            "xbf": xp.astype(BF), "xt": np.ascontiguousarray(xp.T).astype(BF),
            "ncs": ncs, "idx": idx_arrs[c], "slots": slot_arrs[c],
            "ptab": pt_arrs[c],
            "w1e": w1e, "w2e": w2e, "iotax": iotax,
            "ident": identm,
        }
        if not ln_trivial:
            m["badd"] = badd
        if not b1_zero:
            m["b1r"] = np.tile(b1[perm1][None, :], (128, 1)).astype(np.float32)
        if not b2_zero:
            m["b2r"] = np.tile(b2[None, :], (128, 1)).astype(np.float32)
        in_maps.append(m)
    meta = (NW, Woff, SWoff, Wtot, SWtot, nexact, ln_trivial, b1_zero, b2_zero)
    return meta, in_maps


def kernel(**inputs):
    _install_ntff_hook()
    from concourse.bass_utils import run_bass_kernel_spmd

    meta, in_maps = _host_prep(inputs)
    NW, Woff, SWoff, Wtot, SWtot, nexact, ln_trivial, b1_zero, b2_zero = meta
    key = (Wtot, SWtot, ln_trivial, b1_zero, b2_zero, NW.tobytes(),
           nexact.tobytes())
    if key not in _cache:
        _cache[key] = _build(NW, Woff, SWoff, Wtot, SWtot, nexact,
                             ln_trivial, b1_zero, b2_zero)
    nc = _cache[key]

    trace = bool(int(__import__("os").environ.get("KERNEL_TRACE", "0")))
    res = run_bass_kernel_spmd(nc, in_maps, core_ids=list(range(NCORE)),
                               trace=trace)
    kernel.last_exec_time_ns = res.exec_time_ns
    out = np.concatenate([res.results[c]["out"][:NLOC] for c in range(NCORE)], 0)
    return out.astype(np.float32)


kernel.last_exec_time_ns = None
